# revision 34
# baseline (speedup 1.0000x reference)
"""Trainium2 Bass kernel for nn_Align_54279796687162 (sparse_attention).

Pure data parallel: one sample per NeuronCore (B=8 over 8 cores).
Per-core layout: activations channel-major [C(partitions), n = h*128 + w].
bf16 matmul inputs, f32 PSUM accumulation.

v2 structure:
 - Phase A: conv3x3 in 12-row slabs; cb^T / cf^T via DMA-transpose (xbar);
   energy accumulated from transposed tiles; shunts of cb/cf computed here
   (DVE idle during conv); softmax -> attn halves -> on-device rank-16
   correction matrices A = scale*(W @ attn) so xf is never materialized.
 - Region 2 (one scheduling scope): qkv from cb + A@cf corrections (q/k
   spilled to DRAM, reloaded as padded windows for the depthwise conv),
   depthwise 3x3 (groups split tensor/DVE), pointwise, axial attention,
   final gating - all interleaved by Tile.

Self-contained: hardcodes shapes, builds the Bass/Tile graph, shards inputs,
runs via run_bass_kernel_spmd on cores 0-7, gathers the full output.
"""

import numpy as np
import ml_dtypes

import concourse.bass as bass
import concourse.mybir as mybir
import concourse.tile as tile
from concourse import bacc
from concourse.bass_utils import run_bass_kernel_spmd

BF = mybir.dt.bfloat16
F32 = mybir.dt.float32
AF = mybir.ActivationFunctionType
ALU = mybir.AluOpType
AX = mybir.AxisListType

H = W = 128
N = H * W            # 16384
BL = 512             # block size (4 rows * 128)
CH = 4               # chunks
SCALE = 0.25         # KD ** -0.5
PST = 132            # padded row stride for q/k/v (DW conv layout)
PSZ = PST * 130      # padded tensor size per partition
SLAB = 12            # conv slab rows

# depthwise groups 0..DW_TENSOR_GROUPS-1 (of q,k,v0,v1) run as diag matmuls
# on the tensor engine; the rest run as DVE FMA chains.
DW_TENSOR_GROUPS = 3

# bias column map in the packed [128, 20] f32 bias tile
B_CCAM, B_ENC, B_Q, B_K, B_V, B_DW, B_PW, B_ROW, B_COL, B_PROJ3 = (
    0, 2, 3, 4, 5, 7, 11, 13, 15, 17)

_CACHE = {}


def _ap(base, extra_off, free_dims):
    """Build an AP from a tile's base AP with custom free dims."""
    b = base[:]
    return bass.AP(b.tensor, b.offset + extra_off, [list(b.ap[0])] + free_dims)


def build_graph(scale_ccam: float):
    nc = bacc.Bacc(None, target_bir_lowering=False)

    xb = nc.dram_tensor("xb", [128, N], F32, kind="ExternalInput")
    w3t = nc.dram_tensor("w3t", [128, 9 * 256], BF, kind="ExternalInput")
    wenc = nc.dram_tensor("wenc", [128, 32], BF, kind="ExternalInput")
    wq = nc.dram_tensor("wq", [128, 256], BF, kind="ExternalInput")
    wk = nc.dram_tensor("wk", [128, 256], BF, kind="ExternalInput")
    wv = nc.dram_tensor("wv", [128, 512], BF, kind="ExternalInput")
    wqs = nc.dram_tensor("wqs", [128, 256], BF, kind="ExternalInput")
    wks = nc.dram_tensor("wks", [128, 256], BF, kind="ExternalInput")
    wvs = nc.dram_tensor("wvs", [128, 512], BF, kind="ExternalInput")
    dwd = nc.dram_tensor("dwd", [128, 36 * 128], BF, kind="ExternalInput")
    wpw = nc.dram_tensor("wpw", [128, 4 * 256], BF, kind="ExternalInput")
    wrow = nc.dram_tensor("wrow", [128, 512], BF, kind="ExternalInput")
    wcol = nc.dram_tensor("wcol", [128, 512], BF, kind="ExternalInput")
    wproj = nc.dram_tensor("wproj", [128, 512], BF, kind="ExternalInput")
    post = nc.dram_tensor("post", [16, 4 * 512], BF, kind="ExternalInput")
    interpm = nc.dram_tensor("interpm", [16, 128], BF, kind="ExternalInput")
    identb = nc.dram_tensor("identb", [128, 128], BF, kind="ExternalInput")
    identf = nc.dram_tensor("identf", [128, 128], F32, kind="ExternalInput")
    onesb = nc.dram_tensor("onesb", [128, 1], BF, kind="ExternalInput")
    biases = nc.dram_tensor("biases", [128, 20], F32, kind="ExternalInput")
    dwsc = nc.dram_tensor("dwsc", [128, 36], F32, kind="ExternalInput")

    cb_dram = nc.dram_tensor("cb_dram", [2, 128, N], BF, kind="Internal")
    cf_dram = nc.dram_tensor("cf_dram", [16, N], BF, kind="Internal")
    qk_dram = nc.dram_tensor("qk_dram", [2, 128, N], BF, kind="Internal")
    qo_dram = nc.dram_tensor("qo_dram", [2, 128, N], BF, kind="Internal")
    out = nc.dram_tensor("out", [256, N], F32, kind="ExternalOutput")

    # conv slab row-starts: 10 slabs of 12 rows + 1 slab of 8
    slabs = [(s * SLAB, SLAB) for s in range(10)] + [(120, 8)]
    taps = [(1, 1), (0, 1), (2, 1), (1, 0), (1, 2),
            (0, 0), (0, 2), (2, 0), (2, 2)]

    with tile.TileContext(nc) as tc:
      with tc.tile_pool(name="cst", bufs=1) as cst:
        wenc_s = cst.tile([128, 32], BF)
        wq_s = cst.tile([128, 256], BF)
        wk_s = cst.tile([128, 256], BF)
        wv_s = cst.tile([128, 512], BF)
        wqs_s = cst.tile([128, 256], BF)
        wks_s = cst.tile([128, 256], BF)
        wvs_s = cst.tile([128, 512], BF)
        wpw_s = cst.tile([128, 4 * 256], BF)
        wrow_s = cst.tile([128, 512], BF)
        wcol_s = cst.tile([128, 512], BF)
        wproj_s = cst.tile([128, 512], BF)
        post_s = cst.tile([16, 4 * 512], BF)
        interp_s = cst.tile([16, 128], BF)
        idb_s = cst.tile([128, 128], BF)
        ones_s = cst.tile([128, 1], BF)
        bia_s = cst.tile([128, 20], F32)
        dwsc_s = cst.tile([128, 36], F32)
        for t, d in [(wenc_s, wenc), (wq_s, wq), (wk_s, wk),
                     (wv_s, wv), (wqs_s, wqs), (wks_s, wks), (wvs_s, wvs),
                     (wpw_s, wpw), (wrow_s, wrow),
                     (wcol_s, wcol), (wproj_s, wproj), (post_s, post),
                     (interp_s, interpm), (idb_s, identb),
                     (ones_s, onesb), (bia_s, biases), (dwsc_s, dwsc)]:
            nc.sync.dma_start(t[:], d[:])

        # persistent small tensors produced in phase A, consumed later
        xfs_row = [cst.tile([128, 512], BF, tag=f"xfsr{h}", name=f"xfsr{h}")
                   for h in range(2)]
        xfs_col = [cst.tile([128, 512], F32, tag=f"xfsc{h}", name=f"xfsc{h}")
                   for h in range(2)]
        cfs_row = cst.tile([16, 512], F32)
        cfs_col = cst.tile([16, 512], F32)
        cfs_row_b = cst.tile([16, 512], BF)
        cfs_col_b = cst.tile([16, 512], BF)
        at_h = [cst.tile([128, 16], BF, tag=f"at{h}", name=f"at{h}")
                for h in range(2)]
        # correction matrices A^T [16, 128]: q, k, v0, v1 (normal + shunt).
        # normal set packed at partition offsets 32*i for tile_position use.
        A4n = cst.tile([128, 128], BF)
        A_s = [cst.tile([16, 128], BF, tag=f"As{i}", name=f"As{i}")
               for i in range(4)]
        xproj = {(d_, t_): cst.tile([128, 512], BF, tag=f"xp{d_}{t_}",
                                    name=f"xp{d_}{t_}")
                 for d_ in range(2) for t_ in range(2)}

        # =========================================================
        # Phase A: conv3x3 slabs; cb^T/cf^T via DMA transpose;
        # energy; shunts; softmax; A matrices
        # =========================================================
        with (
            tc.tile_pool(name="pa", bufs=1) as pa,
            tc.tile_pool(name="pasl", bufs=4) as pasl,
            tc.tile_pool(name="par", bufs=3) as par,
            tc.tile_pool(name="pamm", bufs=2, space="PSUM") as pamm,
            tc.tile_pool(name="pacf", bufs=2, space="PSUM") as pacf,
            tc.tile_pool(name="pae", bufs=1, space="PSUM") as pae,
            tc.tile_pool(name="pasm", bufs=1, space="PSUM") as pasm,
        ):
            xpad = pa.tile([128, 130 * 130], BF)
            w3_s = pa.tile([128, 9 * 256], BF)
            idf_s = pa.tile([128, 128], F32)
            nc.sync.dma_start(w3_s[:], w3t[:])
            nc.sync.dma_start(idf_s[:], identf[:])

            # pad borders only; interior filled by strided cast-DMA
            nc.vector.memset(_ap(xpad, 0, [[1, 130]]), 0.0)
            nc.vector.memset(_ap(xpad, 129 * 130, [[1, 130]]), 0.0)
            nc.vector.memset(_ap(xpad, 129, [[130, 129], [1, 2]]), 0.0)
            for rc in range(4):
                nc.gpsimd.dma_start(
                    _ap(xpad, 131 + rc * 32 * 130, [[130, 32], [1, 128]]),
                    xb[:, rc * 4096:(rc + 1) * 4096])

            e_ps = pae.tile([16, 256], F32)

            first_mm = [True]
            pend = []   # (cbT, cfT, srows) pending energy MMs, 1-slab delay

            def emit_energy(last):
                cbT, cfT, srows = pend.pop(0)
                for j in range(srows):
                    for half in range(2):
                        nc.tensor.matmul(
                            e_ps[:, half * 128:(half + 1) * 128],
                            cfT[:, j, :], cbT[half][:, j, :],
                            start=first_mm[0],
                            stop=(last and j == srows - 1 and half == 1))
                        first_mm[0] = False

            for si, (s0, srows) in enumerate(slabs):
                # --- conv3x3 for this slab, both halves ---
                grp = [(r0, min(3, s0 + srows - r0))
                       for r0 in range(s0, s0 + srows, 3)]
                cbs = [pasl.tile([128, SLAB * 128], BF, tag=f"cbs{h}",
                                 name=f"cbs{h}") for h in range(2)]
                for half in range(2):
                    for g0 in range(0, len(grp), 2):
                        pg = grp[g0:g0 + 2]
                        ps = pamm.tile([128, 1024], F32, tag="amm")
                        for t9 in range(9):
                            ky, kx = divmod(t9, 3)
                            for j, (r0, nr) in enumerate(pg):
                                rhs = _ap(xpad, (r0 + ky) * 130 + kx,
                                          [[1, nr * 130 - 2]])
                                nc.tensor.matmul(
                                    _ap(ps, j * 512, [[1, nr * 130 - 2]]),
                                    w3_s[:, t9 * 256 + half * 128:
                                         t9 * 256 + half * 128 + 128],
                                    rhs, start=(t9 == 0), stop=(t9 == 8))
                        nrows = sum(nr for _, nr in pg)
                        if len(pg) == 2:
                            src = _ap(ps, 0, [[512, 2], [130, pg[0][1]],
                                              [1, 128]])
                        else:
                            src = _ap(ps, 0, [[130, pg[0][1]], [1, 128]])
                        lr = pg[0][0] - s0
                        nc.scalar.activation(
                            cbs[half][:, lr * 128:(lr + nrows) * 128],
                            src, AF.Relu,
                            bias=bia_s[:, B_CCAM + half:B_CCAM + half + 1])
                    nc.sync.dma_start(
                        cb_dram[half, :, s0 * 128:(s0 + srows) * 128],
                        cbs[half][:, 0:srows * 128])

                # --- cf for this slab: relu(wenc @ cb + b_enc) ---
                nbl = (srows * 128) // 512
                cfsl = pasl.tile([16, SLAB * 128], BF, tag="cfsl")
                for b in range(nbl):
                    ps = pacf.tile([16, 512], F32, tag="acf")
                    for half in range(2):
                        nc.tensor.matmul(
                            ps[:], wenc_s[:, half * 16:half * 16 + 16],
                            cbs[half][:, b * 512:(b + 1) * 512],
                            start=(half == 0), stop=(half == 1))
                    nc.scalar.activation(
                        cfsl[:, b * 512:(b + 1) * 512], ps[:], AF.Relu,
                        bias=bia_s[:16, B_ENC:B_ENC + 1])
                nc.sync.dma_start(
                    cf_dram[:, s0 * 128:(s0 + srows) * 128],
                    cfsl[:, 0:srows * 128])

                # --- transposes via DMA xbar ---
                cbT = [pasl.tile([128, SLAB, 128], BF, tag=f"cbT{h}",
                                 name=f"cbT{h}") for h in range(2)]
                cfT = pasl.tile([128, SLAB, 16], BF, tag="cfT")
                for half in range(2):
                    nc.sync.dma_start(cbT[half][:, 0:srows, :],
                                      cbs[half][:, 0:srows * 128],
                                      transpose=True)
                nc.sync.dma_start(cfT[:, 0:srows, :],
                                  cfsl[:, 0:srows * 128], transpose=True)

                # --- energy accumulation, pipelined one slab behind ---
                pend.append((cbT, cfT, srows))
                if si >= 1:
                    emit_energy(last=False)

                # --- shunts of cb (into xfs_*) and cf (into cfs_*) ---
                for b4 in range(srows // 4):
                    b = (s0 // 4) + b4
                    lr = b4 * 4
                    ci = b // 8
                    for half in range(2):
                        with nc.allow_low_precision(reason="bf16 shunt sums"):
                            src = _ap(cbs[half], lr * 128,
                                      [[1, 4], [128, 4], [4, 32]])
                            dst = _ap(xfs_row[half], 4 * b,
                                      [[128, 4], [1, 4]])
                            nc.vector.tensor_reduce(dst, src, axis=AX.X,
                                                    op=ALU.add)
                        part = par.tile([128, 128], F32, tag=f"cp{half}",
                                        name=f"cp{half}", bufs=2)
                        src = _ap(cbs[half], lr * 128, [[1, 128], [128, 4]])
                        nc.vector.tensor_reduce(part[:], src, axis=AX.X,
                                                op=ALU.add)
                        dstc = xfs_col[half][:, ci * 128:(ci + 1) * 128]
                        if b % 8 == 0:
                            nc.gpsimd.tensor_copy(dstc, part[:])
                        else:
                            nc.gpsimd.tensor_tensor(dstc, dstc, part[:],
                                                    ALU.add)
                    # cf shunts
                    src = _ap(cfsl, lr * 128, [[1, 4], [128, 4], [4, 32]])
                    dst = _ap(cfs_row, 4 * b, [[128, 4], [1, 4]])
                    nc.vector.tensor_reduce(dst, src, axis=AX.X, op=ALU.add)
                    partf = par.tile([16, 128], F32, tag="cpf", bufs=2)
                    src = _ap(cfsl, lr * 128, [[1, 128], [128, 4]])
                    nc.vector.tensor_reduce(partf[:], src, axis=AX.X,
                                            op=ALU.add)
                    dstc = cfs_col[:, ci * 128:(ci + 1) * 128]
                    if b % 8 == 0:
                        nc.gpsimd.tensor_copy(dstc, partf[:])
                    else:
                        nc.gpsimd.tensor_tensor(dstc, dstc, partf[:], ALU.add)

            emit_energy(last=True)
            nc.vector.tensor_copy(cfs_row_b[:], cfs_row[:])
            nc.vector.tensor_copy(cfs_col_b[:], cfs_col[:])

            # --- CCAM softmax: attn = softmax(-energy) over K=16 ---
            e_sb = pa.tile([16, 256], F32)
            nc.scalar.activation(e_sb[:], e_ps[:], AF.Copy)
            for half in range(2):
                tps = pasm.tile([128, 16], F32, tag="sm")
                nc.tensor.transpose(
                    tps[:], e_sb[:, half * 128:(half + 1) * 128],
                    idf_s[:16, :16])
                e_c = par.tile([128, 16], F32, tag="ec")
                nc.vector.tensor_copy(e_c[:], tps[:])
                mn = par.tile([128, 1], F32, tag="mn")
                nc.vector.tensor_reduce(mn[:], e_c[:], axis=AX.X, op=ALU.min)
                ex = par.tile([128, 16], F32, tag="ex")
                nc.scalar.activation(ex[:], e_c[:], AF.Exp,
                                     bias=mn[:], scale=-1.0)
                sm = par.tile([128, 1], F32, tag="smv")
                nc.vector.tensor_reduce(sm[:], ex[:], axis=AX.X, op=ALU.add)
                rc = par.tile([128, 1], F32, tag="rc")
                nc.vector.reciprocal(rc[:], sm[:])
                nc.vector.tensor_scalar(at_h[half][:], ex[:], rc[:],
                                        float(scale_ccam), ALU.mult, ALU.mult)

            # --- A^T matrices: A^T = at^T @ W^T (both halves accumulated)
            # normal set lands in A4n rows 32*i..32*i+15 (i: q,k,v0,v1)
            for (dsts, wt, nt) in [(("n", 0), wq_s, 1),
                                   (("n", 1), wk_s, 1),
                                   (("n", 2), wv_s, 2),
                                   ((A_s[0],), wqs_s, 1),
                                   ((A_s[1],), wks_s, 1),
                                   ((A_s[2], A_s[3]), wvs_s, 2)]:
                for mt in range(nt):
                    ps = pasm.tile([16, 128], F32, tag="sm")
                    for half in range(2):
                        nc.tensor.matmul(
                            ps[:], at_h[half][:],
                            wt[:, (half * nt + mt) * 128:
                               (half * nt + mt) * 128 + 128],
                            start=(half == 0), stop=(half == 1))
                    if dsts[0] == "n":
                        i4 = dsts[1] + mt
                        nc.scalar.activation(
                            A4n[32 * i4:32 * i4 + 16, :], ps[:], AF.Copy)
                    else:
                        nc.scalar.activation(dsts[mt][:], ps[:], AF.Copy)

        # =========================================================
        # Region 2: qkv, depthwise+pointwise, axial attn, final
        # =========================================================
        with (
            tc.tile_pool(name="pv", bufs=1) as pv,
            tc.tile_pool(name="pb", bufs=1) as pb,
            tc.tile_pool(name="pbr", bufs=3) as pbr,
        ):
            pqk_cm = tc.tile_pool(name="pqk", bufs=2, space="PSUM")
            pqk = pqk_cm.__enter__()
            v_sb = [pv.tile([128, PSZ], BF, tag=f"v{h}", name=f"v{h}")
                    for h in range(2)]
            for t_ in v_sb:
                # zero only the pad cells: rows 0/129, cols {0,1,130,131}
                nc.gpsimd.memset(_ap(t_, 0, [[129 * PST, 2], [1, PST]]), 0.0)
                nc.gpsimd.memset(
                    _ap(t_, PST, [[PST, 128], [130, 2], [1, 2]]), 0.0)

            # ---- qkv production: 8 slabs of 2048 cols ----
            # q/k go to DRAM (plain layout); v stays resident (padded).
            # each weight loaded once per 4 matmuls; the 4 rank-16
            # corrections run concurrently via tile_position row groups.
            for pr in range(8):
                cbi = [pbr.tile([128, 2048], BF, tag=f"cbi{h}",
                                name=f"cbi{h}", bufs=2) for h in range(2)]
                cfi = pbr.tile([128, 2048], BF, tag="cfi", bufs=1)
                sl = slice(pr * 2048, (pr + 1) * 2048)
                nc.sync.dma_start(cbi[0][:], cb_dram[0, :, sl])
                nc.sync.dma_start(cbi[1][:], cb_dram[1, :, sl])
                for i4 in range(4):
                    nc.sync.dma_start(cfi[32 * i4:32 * i4 + 16, :],
                                      cf_dram[:, sl])

                for (ti, wt, i4s, bc, nt) in [
                        (0, wq_s, (0,), B_Q, 1),
                        (1, wk_s, (1,), B_K, 1),
                        (2, wv_s, (2, 3), B_V, 2)]:
                    for mt in range(nt):
                        i4 = i4s[mt]
                        ps = pqk.tile([128, 2048], F32, tag="qmm")
                        for kh in range(2):
                            for j in range(4):
                                nc.tensor.matmul(
                                    ps[:, j * 512:(j + 1) * 512],
                                    wt[:, (kh * nt + mt) * 128:
                                       (kh * nt + mt) * 128 + 128],
                                    cbi[kh][:, j * 512:(j + 1) * 512],
                                    start=(kh == 0), stop=False)
                        for j in range(4):
                            nc.tensor.matmul(
                                ps[:, j * 512:(j + 1) * 512],
                                A4n[32 * i4:32 * i4 + 16, :],
                                cfi[32 * i4:32 * i4 + 16,
                                    j * 512:(j + 1) * 512],
                                start=False, stop=True,
                                tile_position=(32 * i4, 0))
                        if ti < 2:
                            qkst = pbr.tile([128, 2048], BF, tag="qkst",
                                            bufs=1)
                            nc.scalar.activation(
                                qkst[:], ps[:], AF.Identity,
                                bias=bia_s[:, bc + mt:bc + mt + 1])
                            nc.sync.dma_start(qk_dram[ti, :, sl], qkst[:])
                        else:
                            pdst = _ap(v_sb[mt], (16 * pr + 1) * PST + 2,
                                       [[PST, 16], [1, 128]])
                            nc.scalar.activation(
                                pdst, ps[:], AF.Identity,
                                bias=bia_s[:, bc + mt:bc + mt + 1])

            pqk_cm.__exit__(None, None, None)
            pcm_cm = tc.tile_pool(name="pcm", bufs=2, space="PSUM")
            pcm = pcm_cm.__enter__()

            # ---- DVE depthwise FMA chains (fills V during qkv/C1) ----
            # per 24-row chunk: 9-tap STT chain into acc, then one
            # bias+relu extraction of the whole chunk (strips pads).
            dve_dw = {}  # t -> list of (c0, crows, chunk-output tile)

            def dve_chain(t, c0):
                vsrc = v_sb[t - 2]
                crows = min(24, 128 - c0)
                start = (c0 + 1) * PST + 2
                nn = crows * PST - 4
                acc = pbr.tile([128, 24 * PST], BF, tag=f"dacc{t}",
                               name=f"dacc{t}", bufs=1)
                acc_ap = _ap(acc, 0, [[1, nn]])
                nc.vector.tensor_scalar(
                    acc_ap, _ap(vsrc, start, [[1, nn]]),
                    dwsc_s[:, t * 9 + 4:t * 9 + 5], None, ALU.mult)
                for (ky, kx) in taps[1:]:
                    tap9 = ky * 3 + kx
                    delta = (ky - 1) * PST + (kx - 1)
                    src = _ap(vsrc, start + delta, [[1, nn]])
                    nc.vector.scalar_tensor_tensor(
                        acc_ap, src,
                        dwsc_s[:, t * 9 + tap9:t * 9 + tap9 + 1],
                        acc_ap, ALU.mult, ALU.add)
                return crows, acc

            def dve_extract(t, c0, crows, acc, outs):
                # extract in 12-row pieces (smaller resident footprint)
                for s12 in range(0, crows, 12):
                    rows = min(12, crows - s12)
                    dwc = pbr.tile([128, 12 * 128], BF, tag=f"dwc{t}",
                                   name=f"dwc{t}", bufs=2)
                    nc.vector.tensor_scalar(
                        dwc[:, 0:rows * 128],
                        _ap(acc, s12 * PST, [[PST, rows], [1, 128]]),
                        bia_s[:, B_DW + t:B_DW + t + 1], 0.0,
                        ALU.add, ALU.max)
                    outs.append((c0 + s12, dwc))

            for t in range(DW_TENSOR_GROUPS, 4):
                pend_dw = None
                outs = []
                for c0 in range(0, 128, 24):
                    crows, acc = dve_chain(t, c0)
                    if pend_dw is not None:
                        p0, pcr, pacc = pend_dw
                        dve_extract(t, p0, pcr, pacc, outs)
                    pend_dw = (c0, crows, acc)
                p0, pcr, pacc = pend_dw
                dve_extract(t, p0, pcr, pacc, outs)
                dve_dw[t] = outs

            # ---- C1 axial attention ----
            xfs_cb = [pb.tile([128, 512], BF, tag=f"xfcb{h}",
                              name=f"xfcb{h}") for h in range(2)]
            for hh in range(2):
                nc.gpsimd.tensor_copy(xfs_cb[hh][:], xfs_col[hh][:])
            for d_ in range(2):
                xfs = xfs_row if d_ == 0 else xfs_cb
                cfs_b = cfs_row_b if d_ == 0 else cfs_col_b
                qs_att = pb.tile([128, 512], BF, tag="qsa", bufs=2)
                ks_att = pb.tile([128, 512], BF, tag="ksa", bufs=2)
                vs_att = [pb.tile([128, 512], BF, tag=f"vsa{h}",
                                  name=f"vsa{h}", bufs=2) for h in range(2)]
                for (dst, wt, As_i, bc, nt, pidx) in [
                        ([qs_att], wqs_s, (0,), B_Q, 1, 2 * d_),
                        ([ks_att], wks_s, (1,), B_K, 1, 2 * d_ + 1),
                        (vs_att, wvs_s, (2, 3), B_V, 2, None)]:
                    for mt in range(nt):
                        ps = pcm.tile([128, 512], F32, tag="cmm", bufs=2)
                        for kh in range(2):
                            nc.tensor.matmul(
                                ps[:],
                                wt[:, (kh * nt + mt) * 128:
                                   (kh * nt + mt) * 128 + 128],
                                xfs[kh][:], start=(kh == 0), stop=False)
                        nc.tensor.matmul(ps[:], A_s[As_i[mt]][:], cfs_b[:],
                                         start=False, stop=(pidx is None))
                        if pidx is not None:
                            for i in range(CH):
                                nc.tensor.matmul(
                                    ps[:, i * 128:(i + 1) * 128],
                                    post_s[:, (pidx * 4 + i) * 128:
                                           (pidx * 4 + i) * 128 + 128],
                                    interp_s[:], start=False, stop=(i == 3))
                        nc.scalar.activation(
                            dst[mt][:], ps[:], AF.Identity,
                            bias=bia_s[:, bc + mt:bc + mt + 1])

                # repack q/k: 4 heads per 32-partition row group
                q_pack = pb.tile([128, 1024], BF, tag="qp", name="qp", bufs=2)
                k_pack = pb.tile([128, 1024], BF, tag="kp", name="kp", bufs=2)
                for g in range(8):
                    po, co = 32 * (g % 4), (g // 4) * 512
                    nc.sync.dma_start(
                        q_pack[po:po + 16, co:co + 512],
                        qs_att[g * 16:(g + 1) * 16, :])
                    nc.sync.dma_start(
                        k_pack[po:po + 16, co:co + 512],
                        ks_att[g * 16:(g + 1) * 16, :])

                # v^T per chunk: [128(pos), i, 256(ch2)]
                vt_s = pb.tile([128, 4, 256], BF, tag="vt", bufs=2)
                for i in range(CH):
                    for hh in range(2):
                        tp = pcm.tile([128, 128], BF, tag="lps", bufs=2)
                        nc.tensor.transpose(
                            tp[:], vs_att[hh][:, i * 128:(i + 1) * 128],
                            idb_s[:])
                        nc.scalar.activation(
                            vt_s[:, i, hh * 128:(hh + 1) * 128], tp[:],
                            AF.Copy)

                xpre = [pb.tile([128, 512], BF, tag=f"xpre{t}",
                                name=f"xpre{t}", bufs=2) for t in range(2)]
                for i in range(CH):
                    for th in range(2):
                        asm_ps = pcm.tile([128, 128], BF, tag="asm", bufs=2)
                        for gg in range(4):
                            g = th * 4 + gg
                            po = 32 * (g % 4)
                            co = (g // 4) * 512
                            sl_gi = slice(co + i * 128, co + i * 128 + 128)
                            l_ps = pcm.tile([128, 128], F32, tag="lps", bufs=2)
                            nc.tensor.matmul(l_ps[:],
                                             k_pack[po:po + 16, sl_gi],
                                             q_pack[po:po + 16, sl_gi],
                                             start=True, stop=True,
                                             tile_position=(po, 0))
                            e_t = pbr.tile([128, 128], BF, tag="et", bufs=2)
                            nc.scalar.activation(e_t[:], l_ps[:], AF.Exp,
                                                 scale=SCALE)
                            av_ps = pcm.tile([128, 33], F32, tag="av", bufs=2)
                            nc.tensor.matmul(
                                av_ps[:, 0:32], e_t[:],
                                vt_s[:, i, g * 32:(g + 1) * 32],
                                start=True, stop=False)
                            nc.tensor.matmul(av_ps[:, 32:33], e_t[:],
                                             ones_s[:], start=False,
                                             stop=True)
                            rcp = pbr.tile([128, 1], F32, tag="rcp")
                            nc.vector.reciprocal(rcp[:], av_ps[:, 32:33])
                            xrn = pbr.tile([128, 32], BF, tag="xrn")
                            nc.scalar.activation(xrn[:], av_ps[:, 0:32],
                                                 AF.Copy, scale=rcp[:])
                            nc.tensor.transpose(
                                asm_ps[gg * 32:(gg + 1) * 32, :], xrn[:],
                                idb_s[:], tile_position=(0, gg * 32))
                        nc.scalar.activation(
                            xpre[th][:, i * 128:(i + 1) * 128], asm_ps[:],
                            AF.Relu)

                wproj_d = wrow_s if d_ == 0 else wcol_s
                bcol = B_ROW if d_ == 0 else B_COL
                for mt in range(2):
                    ps = pcm.tile([128, 512], F32, tag="cmm", bufs=2)
                    for kh in range(2):
                        nc.tensor.matmul(
                            ps[:],
                            wproj_d[:, (kh * 2 + mt) * 128:
                                    (kh * 2 + mt) * 128 + 128],
                            xpre[kh][:], start=(kh == 0), stop=(kh == 1))
                    nc.scalar.activation(
                        xproj[(d_, mt)][:], ps[:], AF.Identity,
                        bias=bia_s[:, bcol + mt:bcol + mt + 1])

            pcm_cm.__exit__(None, None, None)
            pbmm_cm = tc.tile_pool(name="pbmm", bufs=2, space="PSUM")
            pbmm = pbmm_cm.__enter__()
            pe2_cm = tc.tile_pool(name="pe2", bufs=2, space="PSUM")
            pe2 = pe2_cm.__enter__()

            # ---- depthwise 3x3 ----
            dwd_s = pb.tile([128, 36 * 128], BF)
            nc.sync.dma_start(dwd_s[:], dwd[:])
            dblk = [(r0, 3) for r0 in range(0, 126, 3)] + [(126, 2)]

            def dve_chunk_of(r0):
                return r0 // 24

            def emit_c2a(bg):
                # xx = relu(v + bcast(xrow) + bcast(xcol));
                # att = hsig(proj(xx) + b + 3); out = att * qkv2
                xxg = []
                for j in range(4):
                    b = bg * 4 + j
                    xxr = []
                    for half in range(2):
                        xx = pbr.tile([128, BL], BF, tag=f"xx{half}",
                                      name=f"xx{half}", bufs=3)
                        rap = _ap(xproj[(0, half)], b * 16,
                                  [[1, 16], [0, 32]])
                        cap = _ap(xproj[(1, half)], (b // 2) * 32,
                                  [[0, 4], [0, 4], [1, 32]])
                        nc.vector.tensor_tensor(xx[:], rap, cap, ALU.add)
                        vap = _ap(v_sb[half], (4 * b + 1) * PST + 2,
                                  [[PST, 4], [1, 128]])
                        nc.vector.tensor_tensor(xx[:], xx[:], vap, ALU.add)
                        nc.vector.tensor_scalar(xx[:], xx[:], 0.0, None,
                                                ALU.max)
                        xxr.append(xx)
                    xxg.append(xxr)
                for mt in range(2):
                    for jp in range(2):
                        ps = pe2.tile([128, 1024], F32, tag="jps",
                                      name="jps")
                        for kh in range(2):
                            wsl = wproj_s[:, (kh * 2 + mt) * 128:
                                          (kh * 2 + mt) * 128 + 128]
                            for jj in range(2):
                                j = jp * 2 + jj
                                nc.tensor.matmul(
                                    ps[:, jj * 512:(jj + 1) * 512],
                                    wsl, xxg[j][kh][:],
                                    start=(kh == 0), stop=(kh == 1))
                        for jj in range(2):
                            j = jp * 2 + jj
                            b = bg * 4 + j
                            sl = slice(b * BL, (b + 1) * BL)
                            psj = ps[:, jj * 512:(jj + 1) * 512]
                            hs = pbr.tile([128, BL], BF, tag="hs", bufs=2)
                            nc.scalar.activation(
                                hs[:], psj, AF.Relu,
                                bias=bia_s[:, B_PROJ3 + mt:B_PROJ3 + mt + 1])
                            att_t = pbr.tile([128, BL], BF, tag="att",
                                             bufs=2)
                            nc.vector.tensor_scalar(
                                att_t[:], hs[:], 6.0, 1.0 / 6.0,
                                ALU.min, ALU.mult)
                            qo_in = pbr.tile([128, BL], BF, tag="qoin",
                                             bufs=2)
                            nc.sync.dma_start(qo_in[:], qo_dram[mt, :, sl])
                            ob = pbr.tile([128, BL], BF, tag="ob", bufs=2)
                            nc.vector.tensor_tensor(ob[:], att_t[:],
                                                    qo_in[:], ALU.mult)
                            nc.gpsimd.dma_start(
                                out[mt * 128:(mt + 1) * 128, sl], ob[:])

            # tensor groups + pointwise per 2-block group
            for g0 in range(0, len(dblk), 2):
                grp = dblk[g0:g0 + 2]
                r0g = grp[0][0]
                nrows = sum(nr for _, nr in grp)
                dwg = []
                # q/k windows: padded rows r0g .. r0g+nrows+1
                wins = []
                for t in range(min(DW_TENSOR_GROUPS, 2)):
                    win = pbr.tile([128, 8 * PST], BF, tag=f"win{t}",
                                   name=f"win{t}", bufs=2)
                    wrows = nrows + 2
                    # zero pad columns (and edge pad rows)
                    nc.gpsimd.memset(
                        _ap(win, 0, [[PST, wrows], [130, 2], [1, 2]]), 0.0)
                    ia = max(r0g - 1, 0)
                    ib = min(r0g + nrows, 127)
                    if r0g == 0:
                        nc.gpsimd.memset(_ap(win, 2, [[1, 128]]), 0.0)
                    if r0g + nrows > 127:
                        nc.gpsimd.memset(
                            _ap(win, (128 - r0g + 1) * PST + 2,
                                [[1, 128]]), 0.0)
                    nc.sync.dma_start(
                        _ap(win, (ia - (r0g - 1)) * PST + 2,
                            [[PST, ib - ia + 1], [1, 128]]),
                        qk_dram[t, :, ia * 128:(ib + 1) * 128])
                    wins.append(win)

                for t in range(DW_TENSOR_GROUPS):
                    ps = pbmm.tile([128, 1024], F32, tag="bmm")
                    for tt, (ky, kx) in enumerate(taps):
                        tap9 = ky * 3 + kx
                        wsl = dwd_s[:, (t * 9 + tap9) * 128:
                                    (t * 9 + tap9) * 128 + 128]
                        for j, (r0, nr) in enumerate(grp):
                            nn = nr * PST - 4
                            if t < 2:
                                rhs = _ap(wins[t],
                                          (r0 - r0g + ky) * PST + kx + 1,
                                          [[1, nn]])
                            else:
                                rhs = _ap(v_sb[t - 2],
                                          (r0 + ky) * PST + kx + 1,
                                          [[1, nn]])
                            nc.tensor.matmul(
                                _ap(ps, j * 512, [[1, nn]]), wsl, rhs,
                                start=(tt == 0), stop=(tt == 8))
                    dwt = [pbr.tile([128, 384], BF, tag=f"dw{t}{j}",
                                    name=f"dw{t}{j}", bufs=2)
                           for j in range(len(grp))]
                    for j, (r0, nr) in enumerate(grp):
                        nc.scalar.activation(
                            dwt[j][:, 0:nr * 128],
                            _ap(ps, j * 512, [[PST, nr], [1, 128]]),
                            AF.Relu,
                            bias=bia_s[:, B_DW + t:B_DW + t + 1])
                    dwg.append(dwt)
                for t in range(DW_TENSOR_GROUPS, 4):
                    slc = []
                    for j, (r0, nr) in enumerate(grp):
                        b0, dwc = dve_dw[t][r0 // 12]
                        slc.append(dwc[:, (r0 - b0) * 128:
                                       (r0 - b0 + nr) * 128])
                    dwg.append(slc)
                # pointwise
                for mt in range(2):
                    ps = pbmm.tile([128, 1024], F32, tag="bmm", name="pwm")
                    for kt in range(4):
                        wsl = wpw_s[:, kt * 256 + mt * 128:
                                    kt * 256 + mt * 128 + 128]
                        for j, (r0, nr) in enumerate(grp):
                            rhs = (dwg[kt][j][:, 0:nr * 128]
                                   if kt < DW_TENSOR_GROUPS else dwg[kt][j])
                            nc.tensor.matmul(
                                ps[:, j * 512:j * 512 + nr * 128], wsl,
                                rhs, start=(kt == 0), stop=(kt == 3))
                    qo = pbr.tile([128, 768], BF, tag="qo", bufs=1)
                    if len(grp) == 2:
                        src = _ap(ps, 0, [[512, 2], [1, grp[0][1] * 128]])
                    else:
                        src = _ap(ps, 0, [[1, grp[0][1] * 128]])
                    nc.scalar.activation(
                        qo[:, 0:nrows * 128], src, AF.Identity,
                        bias=bia_s[:, B_PW + mt:B_PW + mt + 1])
                    nc.sync.dma_start(
                        qo_dram[mt, :, r0g * 128:(r0g + nrows) * 128],
                        qo[:, 0:nrows * 128])

                # interleave the final gating once its qo rows are written
                pi = g0 // 2
                for bg in range(8):
                    if (16 * bg + 15) // 6 == pi:
                        emit_c2a(bg)

            pe2_cm.__exit__(None, None, None)
            pbmm_cm.__exit__(None, None, None)

    nc.compile()
    return nc


def _interp_matrix():
    s, n = 16, 128
    src = np.clip((np.arange(n) + 0.5) * (s / n) - 0.5, 0.0, s - 1.0)
    i0 = np.floor(src).astype(np.int64)
    i1 = np.minimum(i0 + 1, s - 1)
    w = src - i0
    M = np.zeros((s, n), np.float64)
    np.add.at(M, (i0, np.arange(n)), 1.0 - w)
    np.add.at(M, (i1, np.arange(n)), w)
    return M


def _bf(x):
    return np.ascontiguousarray(np.asarray(x, np.float32).astype(
        ml_dtypes.bfloat16))


def prep_consts(inputs):
    """Host-side layout prep of all weight tensors (shared across cores)."""
    f = {k: np.asarray(v, np.float32) for k, v in inputs.items()}

    w3 = f["w_ccam_b"]                      # [256, 128, 3, 3]
    w3t = np.zeros((128, 9 * 256), np.float32)
    for ky in range(3):
        for kx in range(3):
            t9 = ky * 3 + kx
            w3t[:, t9 * 256:(t9 + 1) * 256] = w3[:, :, ky, kx].T
    wenc = np.zeros((128, 32), np.float32)  # w_enc [16, 256]
    for half in range(2):
        wenc[:, half * 16:(half + 1) * 16] = \
            f["w_enc"][:, half * 128:(half + 1) * 128].T

    def pack_lhsT(wm, nt):
        # wm [out, in]; returns [128, 2*nt*128]: [ci, (kh*nt+mt)*128+co]
        o, cin = wm.shape
        r = np.zeros((128, 2 * nt * 128), np.float32)
        for kh in range(2):
            for mt in range(nt):
                r[:, (kh * nt + mt) * 128:(kh * nt + mt) * 128 + 128] = \
                    wm[mt * 128:(mt + 1) * 128,
                       kh * 128:(kh + 1) * 128].T
        return r

    wq_p = pack_lhsT(f["w_q"], 1)
    wk_p = pack_lhsT(f["w_k"], 1)
    wv_p = pack_lhsT(f["w_v"], 2)
    wrow_p = pack_lhsT(f["w_row"], 2)
    wcol_p = pack_lhsT(f["w_col"], 2)
    wproj_p = pack_lhsT(f["w_proj"], 2)

    wpw_p = np.zeros((128, 4 * 256), np.float32)   # w_pw [256, 512]
    for kt in range(4):
        for mt in range(2):
            wpw_p[:, kt * 256 + mt * 128:kt * 256 + mt * 128 + 128] = \
                f["w_pw"][mt * 128:(mt + 1) * 128,
                          kt * 128:(kt + 1) * 128].T

    dwdg = np.zeros((128, 36 * 128), np.float32)   # w_dw [512,1,3,3]
    ii = np.arange(128)
    for t in range(4):
        for tap9 in range(9):
            ky, kx = divmod(tap9, 3)
            dwdg[ii, (t * 9 + tap9) * 128 + ii] = \
                f["w_dw"][t * 128 + ii, 0, ky, kx]

    post_p = np.zeros((16, 4 * 512), np.float32)
    for pidx, nm in enumerate(["pos_rowq", "pos_rowk", "pos_colq", "pos_colk"]):
        p = f[nm]                                   # [4, 128, 16]
        for i in range(4):
            post_p[:, (pidx * 4 + i) * 128:(pidx * 4 + i) * 128 + 128] = \
                p[i].T                              # [16, 128]

    biases = np.zeros((128, 20), np.float32)
    biases[:, B_CCAM + 0] = f["b_ccam_b"][:128]
    biases[:, B_CCAM + 1] = f["b_ccam_b"][128:]
    biases[:16, B_ENC] = f["b_enc"]
    biases[:, B_Q] = f["b_q"]
    biases[:, B_K] = f["b_k"]
    biases[:, B_V + 0] = f["b_v"][:128]
    biases[:, B_V + 1] = f["b_v"][128:]
    for t in range(4):
        biases[:, B_DW + t] = f["b_dw"][t * 128:(t + 1) * 128]
    biases[:, B_PW + 0] = f["b_pw"][:128]
    biases[:, B_PW + 1] = f["b_pw"][128:]
    biases[:, B_ROW + 0] = f["b_row"][:128]
    biases[:, B_ROW + 1] = f["b_row"][128:]
    biases[:, B_COL + 0] = f["b_col"][:128]
    biases[:, B_COL + 1] = f["b_col"][128:]
    biases[:, B_PROJ3 + 0] = f["b_proj"][:128] + 3.0
    biases[:, B_PROJ3 + 1] = f["b_proj"][128:] + 3.0

    dwsc_p = np.zeros((128, 36), np.float32)
    for t in range(4):
        for tap9 in range(9):
            ky, kx = divmod(tap9, 3)
            dwsc_p[:, t * 9 + tap9] = f["w_dw"][t * 128:(t + 1) * 128,
                                                0, ky, kx]
    return {
        "dwsc": np.ascontiguousarray(dwsc_p),
        "w3t": _bf(w3t), "wenc": _bf(wenc),
        "wq": _bf(wq_p), "wk": _bf(wk_p), "wv": _bf(wv_p),
        "wqs": _bf(wq_p / 32.0), "wks": _bf(wk_p / 32.0),
        "wvs": _bf(wv_p / 32.0),
        "dwd": _bf(dwdg), "wpw": _bf(wpw_p),
        "wrow": _bf(wrow_p), "wcol": _bf(wcol_p), "wproj": _bf(wproj_p),
        "post": _bf(post_p), "interpm": _bf(_interp_matrix()),
        "identb": _bf(np.eye(128)),
        "identf": np.eye(128, dtype=np.float32),
        "onesb": _bf(np.ones((128, 1))),
        "biases": np.ascontiguousarray(biases),
    }


def kernel(**inputs) -> np.ndarray:
    x = np.asarray(inputs["x"], np.float32)          # [8, 128, 128, 128]
    scale = float(np.asarray(inputs["scale_ccam"]).reshape(-1)[0])

    key = round(scale, 9)
    if key not in _CACHE:
        _CACHE[key] = build_graph(scale)
    nc = _CACHE[key]

    consts = prep_consts(inputs)
    in_maps = []
    for core in range(8):
        m = dict(consts)
        m["xb"] = np.ascontiguousarray(x[core].reshape(128, N))
        in_maps.append(m)

    res = run_bass_kernel_spmd(nc, in_maps, core_ids=list(range(8)))
    outs = [res.results[i]["out"].reshape(256, 128, 128) for i in range(8)]
    return np.stack(outs).astype(np.float32)


if __name__ == "__main__":
    rng = np.random.default_rng(0)
    demo = {"x": rng.standard_normal((8, 128, 128, 128), dtype=np.float32)}
    print("kernel module OK")


# revision 35
# speedup vs baseline: 1.0693x; 1.0693x over previous
"""Trainium2 Bass kernel for nn_Align_54279796687162 (sparse_attention).

Pure data parallel: one sample per NeuronCore (B=8 over 8 cores).
Per-core layout: activations channel-major [C(partitions), n = h*128 + w].
bf16 matmul inputs, f32 PSUM accumulation.

v2 structure:
 - Phase A: conv3x3 in 12-row slabs; cb^T / cf^T via DMA-transpose (xbar);
   energy accumulated from transposed tiles; shunts of cb/cf computed here
   (DVE idle during conv); softmax -> attn halves -> on-device rank-16
   correction matrices A = scale*(W @ attn) so xf is never materialized.
 - Region 2 (one scheduling scope): qkv from cb + A@cf corrections (q/k
   spilled to DRAM, reloaded as padded windows for the depthwise conv),
   depthwise 3x3 (groups split tensor/DVE), pointwise, axial attention,
   final gating - all interleaved by Tile.

Self-contained: hardcodes shapes, builds the Bass/Tile graph, shards inputs,
runs via run_bass_kernel_spmd on cores 0-7, gathers the full output.
"""

import numpy as np
import ml_dtypes

import concourse.bass as bass
import concourse.mybir as mybir
import concourse.tile as tile
from concourse import bacc
from concourse.bass_utils import run_bass_kernel_spmd

BF = mybir.dt.bfloat16
F32 = mybir.dt.float32
AF = mybir.ActivationFunctionType
ALU = mybir.AluOpType
AX = mybir.AxisListType

H = W = 128
N = H * W            # 16384
BL = 512             # block size (4 rows * 128)
CH = 4               # chunks
SCALE = 0.25         # KD ** -0.5
PST = 132            # padded row stride for q/k/v (DW conv layout)
PSZ = PST * 130      # padded tensor size per partition
SLAB = 12            # conv slab rows

# depthwise groups 0..DW_TENSOR_GROUPS-1 (of q,k,v0,v1) run as diag matmuls
# on the tensor engine; the rest run as DVE FMA chains.
DW_TENSOR_GROUPS = 3

# bias column map in the packed [128, 20] f32 bias tile
B_CCAM, B_ENC, B_Q, B_K, B_V, B_DW, B_PW, B_ROW, B_COL, B_PROJ3 = (
    0, 2, 3, 4, 5, 7, 11, 13, 15, 17)

_CACHE = {}


def _ap(base, extra_off, free_dims):
    """Build an AP from a tile's base AP with custom free dims."""
    b = base[:]
    return bass.AP(b.tensor, b.offset + extra_off, [list(b.ap[0])] + free_dims)


def build_graph(scale_ccam: float):
    nc = bacc.Bacc(None, target_bir_lowering=False)

    xb = nc.dram_tensor("xb", [128, N], F32, kind="ExternalInput")
    w3t = nc.dram_tensor("w3t", [128, 9 * 256], BF, kind="ExternalInput")
    wenc = nc.dram_tensor("wenc", [128, 32], BF, kind="ExternalInput")
    wq = nc.dram_tensor("wq", [128, 256], BF, kind="ExternalInput")
    wk = nc.dram_tensor("wk", [128, 256], BF, kind="ExternalInput")
    wv = nc.dram_tensor("wv", [128, 512], BF, kind="ExternalInput")
    wqs = nc.dram_tensor("wqs", [128, 256], BF, kind="ExternalInput")
    wks = nc.dram_tensor("wks", [128, 256], BF, kind="ExternalInput")
    wvs = nc.dram_tensor("wvs", [128, 512], BF, kind="ExternalInput")
    dwd = nc.dram_tensor("dwd", [128, 36 * 128], BF, kind="ExternalInput")
    wpw = nc.dram_tensor("wpw", [128, 4 * 256], BF, kind="ExternalInput")
    wrow = nc.dram_tensor("wrow", [128, 512], BF, kind="ExternalInput")
    wcol = nc.dram_tensor("wcol", [128, 512], BF, kind="ExternalInput")
    wproj = nc.dram_tensor("wproj", [128, 512], BF, kind="ExternalInput")
    post = nc.dram_tensor("post", [16, 4 * 512], BF, kind="ExternalInput")
    interpm = nc.dram_tensor("interpm", [16, 128], BF, kind="ExternalInput")
    identb = nc.dram_tensor("identb", [128, 128], BF, kind="ExternalInput")
    identf = nc.dram_tensor("identf", [128, 128], F32, kind="ExternalInput")
    onesb = nc.dram_tensor("onesb", [128, 1], BF, kind="ExternalInput")
    biases = nc.dram_tensor("biases", [128, 20], F32, kind="ExternalInput")
    dwsc = nc.dram_tensor("dwsc", [128, 36], F32, kind="ExternalInput")

    cb_dram = nc.dram_tensor("cb_dram", [2, 128, N], BF, kind="Internal")
    cf_dram = nc.dram_tensor("cf_dram", [16, N], BF, kind="Internal")
    qk_dram = nc.dram_tensor("qk_dram", [2, 128, N], BF, kind="Internal")
    qo_dram = nc.dram_tensor("qo_dram", [2, 128, N], BF, kind="Internal")
    out = nc.dram_tensor("out", [256, N], F32, kind="ExternalOutput")

    # conv slab row-starts: 10 slabs of 12 rows + 1 slab of 8
    slabs = [(s * SLAB, SLAB) for s in range(10)] + [(120, 8)]
    taps = [(1, 1), (0, 1), (2, 1), (1, 0), (1, 2),
            (0, 0), (0, 2), (2, 0), (2, 2)]

    with tile.TileContext(nc) as tc:
      with tc.tile_pool(name="cst", bufs=1) as cst:
        wenc_s = cst.tile([128, 32], BF)
        wq_s = cst.tile([128, 256], BF)
        wk_s = cst.tile([128, 256], BF)
        wv_s = cst.tile([128, 512], BF)
        wqs_s = cst.tile([128, 256], BF)
        wks_s = cst.tile([128, 256], BF)
        wvs_s = cst.tile([128, 512], BF)
        wpw_s = cst.tile([128, 4 * 256], BF)
        wrow_s = cst.tile([128, 512], BF)
        wcol_s = cst.tile([128, 512], BF)
        wproj_s = cst.tile([128, 512], BF)
        post_s = cst.tile([16, 4 * 512], BF)
        interp_s = cst.tile([16, 128], BF)
        idb_s = cst.tile([128, 128], BF)
        ones_s = cst.tile([128, 1], BF)
        bia_s = cst.tile([128, 20], F32)
        dwsc_s = cst.tile([128, 36], F32)
        for t, d in [(wenc_s, wenc), (wq_s, wq), (wk_s, wk),
                     (wv_s, wv), (wqs_s, wqs), (wks_s, wks), (wvs_s, wvs),
                     (wpw_s, wpw), (wrow_s, wrow),
                     (wcol_s, wcol), (wproj_s, wproj), (post_s, post),
                     (interp_s, interpm), (idb_s, identb),
                     (ones_s, onesb), (bia_s, biases), (dwsc_s, dwsc)]:
            nc.sync.dma_start(t[:], d[:])

        # persistent small tensors produced in phase A, consumed later
        xfs_row = [cst.tile([128, 512], BF, tag=f"xfsr{h}", name=f"xfsr{h}")
                   for h in range(2)]
        xfs_col = [cst.tile([128, 512], F32, tag=f"xfsc{h}", name=f"xfsc{h}")
                   for h in range(2)]
        cfs_row = cst.tile([16, 512], F32)
        cfs_col = cst.tile([16, 512], F32)
        cfs_row_b = cst.tile([16, 512], BF)
        cfs_col_b = cst.tile([16, 512], BF)
        at_h = [cst.tile([128, 16], BF, tag=f"at{h}", name=f"at{h}")
                for h in range(2)]
        # correction matrices A^T [16, 128]: q, k, v0, v1 (normal + shunt).
        # normal set packed at partition offsets 32*i for tile_position use.
        A4n = cst.tile([128, 128], BF)
        A_s = [cst.tile([16, 128], BF, tag=f"As{i}", name=f"As{i}")
               for i in range(4)]
        xproj = {(d_, t_): cst.tile([128, 512], BF, tag=f"xp{d_}{t_}",
                                    name=f"xp{d_}{t_}")
                 for d_ in range(2) for t_ in range(2)}

        # =========================================================
        # Phase A: conv3x3 slabs; cb^T/cf^T via DMA transpose;
        # energy; shunts; softmax; A matrices
        # =========================================================
        with (
            tc.tile_pool(name="pa", bufs=1) as pa,
            tc.tile_pool(name="pasl", bufs=4) as pasl,
            tc.tile_pool(name="par", bufs=3) as par,
            tc.tile_pool(name="pamm", bufs=2, space="PSUM") as pamm,
            tc.tile_pool(name="pacf", bufs=2, space="PSUM") as pacf,
            tc.tile_pool(name="pae", bufs=1, space="PSUM") as pae,
            tc.tile_pool(name="pasm", bufs=1, space="PSUM") as pasm,
        ):
            xpad = pa.tile([128, 130 * 130], BF)
            w3_s = pa.tile([128, 9 * 256], BF)
            idf_s = pa.tile([128, 128], F32)
            nc.sync.dma_start(w3_s[:], w3t[:])
            nc.sync.dma_start(idf_s[:], identf[:])

            # pad borders only; interior filled by strided cast-DMA
            nc.vector.memset(_ap(xpad, 0, [[1, 130]]), 0.0)
            nc.vector.memset(_ap(xpad, 129 * 130, [[1, 130]]), 0.0)
            nc.vector.memset(_ap(xpad, 129, [[130, 129], [1, 2]]), 0.0)
            for rc in range(4):
                nc.gpsimd.dma_start(
                    _ap(xpad, 131 + rc * 32 * 130, [[130, 32], [1, 128]]),
                    xb[:, rc * 4096:(rc + 1) * 4096])

            e_ps = pae.tile([16, 256], F32)

            first_mm = [True]
            pend = []   # (cbT, cfT, srows) pending energy MMs, 1-slab delay

            def emit_energy(last):
                cbT, cfT, srows = pend.pop(0)
                for j in range(srows):
                    for half in range(2):
                        nc.tensor.matmul(
                            e_ps[:, half * 128:(half + 1) * 128],
                            cfT[:, j, :], cbT[half][:, j, :],
                            start=first_mm[0],
                            stop=(last and j == srows - 1 and half == 1))
                        first_mm[0] = False

            for si, (s0, srows) in enumerate(slabs):
                # --- conv3x3 for this slab, both halves ---
                grp = [(r0, min(3, s0 + srows - r0))
                       for r0 in range(s0, s0 + srows, 3)]
                cbs = [pasl.tile([128, SLAB * 128], BF, tag=f"cbs{h}",
                                 name=f"cbs{h}") for h in range(2)]
                for half in range(2):
                    for g0 in range(0, len(grp), 2):
                        pg = grp[g0:g0 + 2]
                        ps = pamm.tile([128, 1024], F32, tag="amm")
                        for t9 in range(9):
                            ky, kx = divmod(t9, 3)
                            for j, (r0, nr) in enumerate(pg):
                                rhs = _ap(xpad, (r0 + ky) * 130 + kx,
                                          [[1, nr * 130 - 2]])
                                nc.tensor.matmul(
                                    _ap(ps, j * 512, [[1, nr * 130 - 2]]),
                                    w3_s[:, t9 * 256 + half * 128:
                                         t9 * 256 + half * 128 + 128],
                                    rhs, start=(t9 == 0), stop=(t9 == 8))
                        nrows = sum(nr for _, nr in pg)
                        if len(pg) == 2:
                            src = _ap(ps, 0, [[512, 2], [130, pg[0][1]],
                                              [1, 128]])
                        else:
                            src = _ap(ps, 0, [[130, pg[0][1]], [1, 128]])
                        lr = pg[0][0] - s0
                        nc.scalar.activation(
                            cbs[half][:, lr * 128:(lr + nrows) * 128],
                            src, AF.Relu,
                            bias=bia_s[:, B_CCAM + half:B_CCAM + half + 1])
                    nc.sync.dma_start(
                        cb_dram[half, :, s0 * 128:(s0 + srows) * 128],
                        cbs[half][:, 0:srows * 128])

                # --- cf for this slab: relu(wenc @ cb + b_enc) ---
                nbl = (srows * 128) // 512
                cfsl = pasl.tile([16, SLAB * 128], BF, tag="cfsl")
                for b in range(nbl):
                    ps = pacf.tile([16, 512], F32, tag="acf")
                    for half in range(2):
                        nc.tensor.matmul(
                            ps[:], wenc_s[:, half * 16:half * 16 + 16],
                            cbs[half][:, b * 512:(b + 1) * 512],
                            start=(half == 0), stop=(half == 1))
                    nc.scalar.activation(
                        cfsl[:, b * 512:(b + 1) * 512], ps[:], AF.Relu,
                        bias=bia_s[:16, B_ENC:B_ENC + 1])
                nc.sync.dma_start(
                    cf_dram[:, s0 * 128:(s0 + srows) * 128],
                    cfsl[:, 0:srows * 128])

                # --- transposes via DMA xbar ---
                cbT = [pasl.tile([128, SLAB, 128], BF, tag=f"cbT{h}",
                                 name=f"cbT{h}") for h in range(2)]
                cfT = pasl.tile([128, SLAB, 16], BF, tag="cfT")
                for half in range(2):
                    nc.sync.dma_start(cbT[half][:, 0:srows, :],
                                      cbs[half][:, 0:srows * 128],
                                      transpose=True)
                nc.sync.dma_start(cfT[:, 0:srows, :],
                                  cfsl[:, 0:srows * 128], transpose=True)

                # --- energy accumulation, pipelined one slab behind ---
                pend.append((cbT, cfT, srows))
                if si >= 1:
                    emit_energy(last=False)

                # --- shunts of cb (into xfs_*) and cf (into cfs_*) ---
                for b4 in range(srows // 4):
                    b = (s0 // 4) + b4
                    lr = b4 * 4
                    ci = b // 8
                    for half in range(2):
                        with nc.allow_low_precision(reason="bf16 shunt sums"):
                            src = _ap(cbs[half], lr * 128,
                                      [[1, 4], [128, 4], [4, 32]])
                            dst = _ap(xfs_row[half], 4 * b,
                                      [[128, 4], [1, 4]])
                            nc.vector.tensor_reduce(dst, src, axis=AX.X,
                                                    op=ALU.add)
                        part = par.tile([128, 128], F32, tag=f"cp{half}",
                                        name=f"cp{half}", bufs=2)
                        src = _ap(cbs[half], lr * 128, [[1, 128], [128, 4]])
                        nc.vector.tensor_reduce(part[:], src, axis=AX.X,
                                                op=ALU.add)
                        dstc = xfs_col[half][:, ci * 128:(ci + 1) * 128]
                        if b % 8 == 0:
                            nc.gpsimd.tensor_copy(dstc, part[:])
                        else:
                            nc.gpsimd.tensor_tensor(dstc, dstc, part[:],
                                                    ALU.add)
                    # cf shunts
                    src = _ap(cfsl, lr * 128, [[1, 4], [128, 4], [4, 32]])
                    dst = _ap(cfs_row, 4 * b, [[128, 4], [1, 4]])
                    nc.vector.tensor_reduce(dst, src, axis=AX.X, op=ALU.add)
                    partf = par.tile([16, 128], F32, tag="cpf", bufs=2)
                    src = _ap(cfsl, lr * 128, [[1, 128], [128, 4]])
                    nc.vector.tensor_reduce(partf[:], src, axis=AX.X,
                                            op=ALU.add)
                    dstc = cfs_col[:, ci * 128:(ci + 1) * 128]
                    if b % 8 == 0:
                        nc.gpsimd.tensor_copy(dstc, partf[:])
                    else:
                        nc.gpsimd.tensor_tensor(dstc, dstc, partf[:], ALU.add)

            emit_energy(last=True)
            nc.vector.tensor_copy(cfs_row_b[:], cfs_row[:])
            nc.vector.tensor_copy(cfs_col_b[:], cfs_col[:])

            # --- CCAM softmax: attn = softmax(-energy) over K=16 ---
            e_sb = pa.tile([16, 256], F32)
            nc.scalar.activation(e_sb[:], e_ps[:], AF.Copy)
            for half in range(2):
                tps = pasm.tile([128, 16], F32, tag="sm")
                nc.tensor.transpose(
                    tps[:], e_sb[:, half * 128:(half + 1) * 128],
                    idf_s[:16, :16])
                e_c = par.tile([128, 16], F32, tag="ec")
                nc.vector.tensor_copy(e_c[:], tps[:])
                mn = par.tile([128, 1], F32, tag="mn")
                nc.vector.tensor_reduce(mn[:], e_c[:], axis=AX.X, op=ALU.min)
                ex = par.tile([128, 16], F32, tag="ex")
                nc.scalar.activation(ex[:], e_c[:], AF.Exp,
                                     bias=mn[:], scale=-1.0)
                sm = par.tile([128, 1], F32, tag="smv")
                nc.vector.tensor_reduce(sm[:], ex[:], axis=AX.X, op=ALU.add)
                rc = par.tile([128, 1], F32, tag="rc")
                nc.vector.reciprocal(rc[:], sm[:])
                nc.vector.tensor_scalar(at_h[half][:], ex[:], rc[:],
                                        float(scale_ccam), ALU.mult, ALU.mult)

            # --- A^T matrices: A^T = at^T @ W^T (both halves accumulated)
            # normal set lands in A4n rows 32*i..32*i+15 (i: q,k,v0,v1)
            for (dsts, wt, nt) in [(("n", 0), wq_s, 1),
                                   (("n", 1), wk_s, 1),
                                   (("n", 2), wv_s, 2),
                                   ((A_s[0],), wqs_s, 1),
                                   ((A_s[1],), wks_s, 1),
                                   ((A_s[2], A_s[3]), wvs_s, 2)]:
                for mt in range(nt):
                    ps = pasm.tile([16, 128], F32, tag="sm")
                    for half in range(2):
                        nc.tensor.matmul(
                            ps[:], at_h[half][:],
                            wt[:, (half * nt + mt) * 128:
                               (half * nt + mt) * 128 + 128],
                            start=(half == 0), stop=(half == 1))
                    if dsts[0] == "n":
                        i4 = dsts[1] + mt
                        nc.scalar.activation(
                            A4n[32 * i4:32 * i4 + 16, :], ps[:], AF.Copy)
                    else:
                        nc.scalar.activation(dsts[mt][:], ps[:], AF.Copy)

        # =========================================================
        # Region 2: qkv, depthwise+pointwise, axial attn, final
        # =========================================================
        with (
            tc.tile_pool(name="pv", bufs=1) as pv,
            tc.tile_pool(name="pb", bufs=1) as pb,
            tc.tile_pool(name="pbr", bufs=3) as pbr,
        ):
            pqk_cm = tc.tile_pool(name="pqk", bufs=2, space="PSUM")
            pqk = pqk_cm.__enter__()
            v_sb = [pv.tile([128, PSZ], BF, tag=f"v{h}", name=f"v{h}")
                    for h in range(2)]
            for t_ in v_sb:
                # zero only the pad cells: rows 0/129, cols {0,1,130,131}
                nc.gpsimd.memset(_ap(t_, 0, [[129 * PST, 2], [1, PST]]), 0.0)
                nc.gpsimd.memset(
                    _ap(t_, PST, [[PST, 128], [130, 2], [1, 2]]), 0.0)

            # ---- qkv production: 8 slabs of 2048 cols ----
            # q/k go to DRAM (plain layout); v stays resident (padded).
            # each weight loaded once per 4 matmuls; the 4 rank-16
            # corrections run concurrently via tile_position row groups.
            for pr in range(8):
                cbi = [pbr.tile([128, 2048], BF, tag=f"cbi{h}",
                                name=f"cbi{h}", bufs=2) for h in range(2)]
                cfi = pbr.tile([128, 2048], BF, tag="cfi", bufs=2)
                sl = slice(pr * 2048, (pr + 1) * 2048)
                nc.sync.dma_start(cbi[0][:], cb_dram[0, :, sl])
                nc.sync.dma_start(cbi[1][:], cb_dram[1, :, sl])
                for i4 in range(4):
                    nc.sync.dma_start(cfi[32 * i4:32 * i4 + 16, :],
                                      cf_dram[:, sl])

                for (ti, wt, i4s, bc, nt) in [
                        (0, wq_s, (0,), B_Q, 1),
                        (1, wk_s, (1,), B_K, 1),
                        (2, wv_s, (2, 3), B_V, 2)]:
                    for mt in range(nt):
                        i4 = i4s[mt]
                        ps = pqk.tile([128, 2048], F32, tag="qmm")
                        for kh in range(2):
                            for j in range(4):
                                nc.tensor.matmul(
                                    ps[:, j * 512:(j + 1) * 512],
                                    wt[:, (kh * nt + mt) * 128:
                                       (kh * nt + mt) * 128 + 128],
                                    cbi[kh][:, j * 512:(j + 1) * 512],
                                    start=(kh == 0), stop=False)
                        for j in range(4):
                            nc.tensor.matmul(
                                ps[:, j * 512:(j + 1) * 512],
                                A4n[32 * i4:32 * i4 + 16, :],
                                cfi[32 * i4:32 * i4 + 16,
                                    j * 512:(j + 1) * 512],
                                start=False, stop=True,
                                tile_position=(32 * i4, 0))
                        if ti < 2:
                            qkst = pbr.tile([128, 2048], BF, tag="qkst",
                                            bufs=1)
                            nc.scalar.activation(
                                qkst[:], ps[:], AF.Identity,
                                bias=bia_s[:, bc + mt:bc + mt + 1])
                            nc.sync.dma_start(qk_dram[ti, :, sl], qkst[:])
                        else:
                            pdst = _ap(v_sb[mt], (16 * pr + 1) * PST + 2,
                                       [[PST, 16], [1, 128]])
                            nc.scalar.activation(
                                pdst, ps[:], AF.Identity,
                                bias=bia_s[:, bc + mt:bc + mt + 1])

            pqk_cm.__exit__(None, None, None)
            pbmm_cm = tc.tile_pool(name="pbmm", bufs=2, space="PSUM")
            pbmm = pbmm_cm.__enter__()
            pcm_cm = tc.tile_pool(name="pcm", bufs=1, space="PSUM")
            pcm = pcm_cm.__enter__()

            # ---- DVE depthwise FMA chains (fills V during qkv/C1) ----
            # per 24-row chunk: 9-tap STT chain into acc, then one
            # bias+relu extraction of the whole chunk (strips pads).
            dve_dw = {}  # t -> list of (c0, crows, chunk-output tile)

            def dve_chain(t, c0):
                vsrc = v_sb[t - 2]
                crows = min(24, 128 - c0)
                start = (c0 + 1) * PST + 2
                nn = crows * PST - 4
                acc = pbr.tile([128, 24 * PST], BF, tag=f"dacc{t}",
                               name=f"dacc{t}", bufs=1)
                acc_ap = _ap(acc, 0, [[1, nn]])
                nc.vector.tensor_scalar(
                    acc_ap, _ap(vsrc, start, [[1, nn]]),
                    dwsc_s[:, t * 9 + 4:t * 9 + 5], None, ALU.mult)
                for (ky, kx) in taps[1:]:
                    tap9 = ky * 3 + kx
                    delta = (ky - 1) * PST + (kx - 1)
                    src = _ap(vsrc, start + delta, [[1, nn]])
                    nc.vector.scalar_tensor_tensor(
                        acc_ap, src,
                        dwsc_s[:, t * 9 + tap9:t * 9 + tap9 + 1],
                        acc_ap, ALU.mult, ALU.add)
                return crows, acc

            def dve_extract(t, c0, crows, acc, outs):
                # extract in 12-row pieces (smaller resident footprint)
                for s12 in range(0, crows, 12):
                    rows = min(12, crows - s12)
                    dwc = pbr.tile([128, 12 * 128], BF, tag=f"dwc{t}",
                                   name=f"dwc{t}", bufs=2)
                    nc.vector.tensor_scalar(
                        dwc[:, 0:rows * 128],
                        _ap(acc, s12 * PST, [[PST, rows], [1, 128]]),
                        bia_s[:, B_DW + t:B_DW + t + 1], 0.0,
                        ALU.add, ALU.max)
                    outs.append((c0 + s12, dwc))

            for t in range(DW_TENSOR_GROUPS, 4):
                pend_dw = None
                outs = []
                for c0 in range(0, 128, 24):
                    crows, acc = dve_chain(t, c0)
                    if pend_dw is not None:
                        p0, pcr, pacc = pend_dw
                        dve_extract(t, p0, pcr, pacc, outs)
                    pend_dw = (c0, crows, acc)
                p0, pcr, pacc = pend_dw
                dve_extract(t, p0, pcr, pacc, outs)
                dve_dw[t] = outs

            # ---- C1 axial attention ----
            xfs_cb = [pb.tile([128, 512], BF, tag=f"xfcb{h}",
                              name=f"xfcb{h}") for h in range(2)]
            for hh in range(2):
                nc.gpsimd.tensor_copy(xfs_cb[hh][:], xfs_col[hh][:])
            for d_ in range(2):
                xfs = xfs_row if d_ == 0 else xfs_cb
                cfs_b = cfs_row_b if d_ == 0 else cfs_col_b
                qs_att = pb.tile([128, 512], BF, tag="qsa", bufs=2)
                ks_att = pb.tile([128, 512], BF, tag="ksa", bufs=2)
                vs_att = [pb.tile([128, 512], BF, tag=f"vsa{h}",
                                  name=f"vsa{h}", bufs=2) for h in range(2)]
                for (dst, wt, As_i, bc, nt, pidx) in [
                        ([qs_att], wqs_s, (0,), B_Q, 1, 2 * d_),
                        ([ks_att], wks_s, (1,), B_K, 1, 2 * d_ + 1),
                        (vs_att, wvs_s, (2, 3), B_V, 2, None)]:
                    for mt in range(nt):
                        ps = pcm.tile([128, 512], F32, tag="cmm")
                        for kh in range(2):
                            nc.tensor.matmul(
                                ps[:],
                                wt[:, (kh * nt + mt) * 128:
                                   (kh * nt + mt) * 128 + 128],
                                xfs[kh][:], start=(kh == 0), stop=False)
                        nc.tensor.matmul(ps[:], A_s[As_i[mt]][:], cfs_b[:],
                                         start=False, stop=(pidx is None))
                        if pidx is not None:
                            for i in range(CH):
                                nc.tensor.matmul(
                                    ps[:, i * 128:(i + 1) * 128],
                                    post_s[:, (pidx * 4 + i) * 128:
                                           (pidx * 4 + i) * 128 + 128],
                                    interp_s[:], start=False, stop=(i == 3))
                        nc.scalar.activation(
                            dst[mt][:], ps[:], AF.Identity,
                            bias=bia_s[:, bc + mt:bc + mt + 1])

                # repack q/k: 4 heads per 32-partition row group
                q_pack = pb.tile([128, 1024], BF, tag="qp", name="qp", bufs=2)
                k_pack = pb.tile([128, 1024], BF, tag="kp", name="kp", bufs=2)
                for g in range(8):
                    po, co = 32 * (g % 4), (g // 4) * 512
                    nc.sync.dma_start(
                        q_pack[po:po + 16, co:co + 512],
                        qs_att[g * 16:(g + 1) * 16, :])
                    nc.sync.dma_start(
                        k_pack[po:po + 16, co:co + 512],
                        ks_att[g * 16:(g + 1) * 16, :])

                # v^T per chunk: [128(pos), i, 256(ch2)]
                vt_s = pb.tile([128, 4, 256], BF, tag="vt", bufs=2)
                for i in range(CH):
                    for hh in range(2):
                        tp = pcm.tile([128, 128], BF, tag="lps")
                        nc.tensor.transpose(
                            tp[:], vs_att[hh][:, i * 128:(i + 1) * 128],
                            idb_s[:])
                        nc.scalar.activation(
                            vt_s[:, i, hh * 128:(hh + 1) * 128], tp[:],
                            AF.Copy)

                xpre = [pb.tile([128, 512], BF, tag=f"xpre{t}",
                                name=f"xpre{t}", bufs=2) for t in range(2)]
                for i in range(CH):
                    for th in range(2):
                        asm_ps = pcm.tile([128, 128], BF, tag="asm")
                        for gg in range(4):
                            g = th * 4 + gg
                            po = 32 * (g % 4)
                            co = (g // 4) * 512
                            sl_gi = slice(co + i * 128, co + i * 128 + 128)
                            l_ps = pcm.tile([128, 128], F32, tag="lps")
                            nc.tensor.matmul(l_ps[:],
                                             k_pack[po:po + 16, sl_gi],
                                             q_pack[po:po + 16, sl_gi],
                                             start=True, stop=True,
                                             tile_position=(po, 0))
                            e_t = pbr.tile([128, 128], BF, tag="et", bufs=2)
                            nc.scalar.activation(e_t[:], l_ps[:], AF.Exp,
                                                 scale=SCALE)
                            av_ps = pcm.tile([128, 33], F32, tag="av")
                            nc.tensor.matmul(
                                av_ps[:, 0:32], e_t[:],
                                vt_s[:, i, g * 32:(g + 1) * 32],
                                start=True, stop=False)
                            nc.tensor.matmul(av_ps[:, 32:33], e_t[:],
                                             ones_s[:], start=False,
                                             stop=True)
                            rcp = pbr.tile([128, 1], F32, tag="rcp")
                            nc.vector.reciprocal(rcp[:], av_ps[:, 32:33])
                            xrn = pbr.tile([128, 32], BF, tag="xrn")
                            nc.scalar.activation(xrn[:], av_ps[:, 0:32],
                                                 AF.Copy, scale=rcp[:])
                            nc.tensor.transpose(
                                asm_ps[gg * 32:(gg + 1) * 32, :], xrn[:],
                                idb_s[:], tile_position=(0, gg * 32))
                        nc.scalar.activation(
                            xpre[th][:, i * 128:(i + 1) * 128], asm_ps[:],
                            AF.Relu)

                wproj_d = wrow_s if d_ == 0 else wcol_s
                bcol = B_ROW if d_ == 0 else B_COL
                for mt in range(2):
                    ps = pcm.tile([128, 512], F32, tag="cmm")
                    for kh in range(2):
                        nc.tensor.matmul(
                            ps[:],
                            wproj_d[:, (kh * 2 + mt) * 128:
                                    (kh * 2 + mt) * 128 + 128],
                            xpre[kh][:], start=(kh == 0), stop=(kh == 1))
                    nc.scalar.activation(
                        xproj[(d_, mt)][:], ps[:], AF.Identity,
                        bias=bia_s[:, bcol + mt:bcol + mt + 1])

            pcm_cm.__exit__(None, None, None)
            pe2_cm = tc.tile_pool(name="pe2", bufs=2, space="PSUM")
            pe2 = pe2_cm.__enter__()

            # ---- depthwise 3x3 ----
            dwd_s = pb.tile([128, 36 * 128], BF)
            nc.sync.dma_start(dwd_s[:], dwd[:])
            dblk = [(r0, 3) for r0 in range(0, 126, 3)] + [(126, 2)]

            def dve_chunk_of(r0):
                return r0 // 24

            def emit_c2a(bg):
                # xx = relu(v + bcast(xrow) + bcast(xcol));
                # att = hsig(proj(xx) + b + 3); out = att * qkv2
                xxg = []
                for j in range(4):
                    b = bg * 4 + j
                    xxr = []
                    for half in range(2):
                        xx = pbr.tile([128, BL], BF, tag=f"xx{half}",
                                      name=f"xx{half}", bufs=3)
                        rap = _ap(xproj[(0, half)], b * 16,
                                  [[1, 16], [0, 32]])
                        cap = _ap(xproj[(1, half)], (b // 2) * 32,
                                  [[0, 4], [0, 4], [1, 32]])
                        nc.vector.tensor_tensor(xx[:], rap, cap, ALU.add)
                        vap = _ap(v_sb[half], (4 * b + 1) * PST + 2,
                                  [[PST, 4], [1, 128]])
                        nc.vector.tensor_tensor(xx[:], xx[:], vap, ALU.add)
                        nc.vector.tensor_scalar(xx[:], xx[:], 0.0, None,
                                                ALU.max)
                        xxr.append(xx)
                    xxg.append(xxr)
                for mt in range(2):
                    for jp in range(2):
                        ps = pe2.tile([128, 1024], F32, tag="jps",
                                      name="jps")
                        for kh in range(2):
                            wsl = wproj_s[:, (kh * 2 + mt) * 128:
                                          (kh * 2 + mt) * 128 + 128]
                            for jj in range(2):
                                j = jp * 2 + jj
                                nc.tensor.matmul(
                                    ps[:, jj * 512:(jj + 1) * 512],
                                    wsl, xxg[j][kh][:],
                                    start=(kh == 0), stop=(kh == 1))
                        for jj in range(2):
                            j = jp * 2 + jj
                            b = bg * 4 + j
                            sl = slice(b * BL, (b + 1) * BL)
                            psj = ps[:, jj * 512:(jj + 1) * 512]
                            hs = pbr.tile([128, BL], BF, tag="hs", bufs=2)
                            nc.scalar.activation(
                                hs[:], psj, AF.Relu,
                                bias=bia_s[:, B_PROJ3 + mt:B_PROJ3 + mt + 1])
                            att_t = pbr.tile([128, BL], BF, tag="att",
                                             bufs=2)
                            nc.vector.tensor_scalar(
                                att_t[:], hs[:], 6.0, 1.0 / 6.0,
                                ALU.min, ALU.mult)
                            qo_in = pbr.tile([128, BL], BF, tag="qoin",
                                             bufs=2)
                            nc.sync.dma_start(qo_in[:], qo_dram[mt, :, sl])
                            ob = pbr.tile([128, BL], BF, tag="ob", bufs=2)
                            nc.vector.tensor_tensor(ob[:], att_t[:],
                                                    qo_in[:], ALU.mult)
                            nc.gpsimd.dma_start(
                                out[mt * 128:(mt + 1) * 128, sl], ob[:])

            # tensor groups + pointwise per 2-block group
            for g0 in range(0, len(dblk), 2):
                grp = dblk[g0:g0 + 2]
                r0g = grp[0][0]
                nrows = sum(nr for _, nr in grp)
                dwg = []
                # q/k windows: padded rows r0g .. r0g+nrows+1
                wins = []
                for t in range(min(DW_TENSOR_GROUPS, 2)):
                    win = pbr.tile([128, 8 * PST], BF, tag=f"win{t}",
                                   name=f"win{t}", bufs=2)
                    wrows = nrows + 2
                    # zero pad columns (and edge pad rows)
                    nc.gpsimd.memset(
                        _ap(win, 0, [[PST, wrows], [130, 2], [1, 2]]), 0.0)
                    ia = max(r0g - 1, 0)
                    ib = min(r0g + nrows, 127)
                    if r0g == 0:
                        nc.gpsimd.memset(_ap(win, 2, [[1, 128]]), 0.0)
                    if r0g + nrows > 127:
                        nc.gpsimd.memset(
                            _ap(win, (128 - r0g + 1) * PST + 2,
                                [[1, 128]]), 0.0)
                    nc.sync.dma_start(
                        _ap(win, (ia - (r0g - 1)) * PST + 2,
                            [[PST, ib - ia + 1], [1, 128]]),
                        qk_dram[t, :, ia * 128:(ib + 1) * 128])
                    wins.append(win)

                for t in range(DW_TENSOR_GROUPS):
                    ps = pbmm.tile([128, 1024], F32, tag="bmm")
                    for tt, (ky, kx) in enumerate(taps):
                        tap9 = ky * 3 + kx
                        wsl = dwd_s[:, (t * 9 + tap9) * 128:
                                    (t * 9 + tap9) * 128 + 128]
                        for j, (r0, nr) in enumerate(grp):
                            nn = nr * PST - 4
                            if t < 2:
                                rhs = _ap(wins[t],
                                          (r0 - r0g + ky) * PST + kx + 1,
                                          [[1, nn]])
                            else:
                                rhs = _ap(v_sb[t - 2],
                                          (r0 + ky) * PST + kx + 1,
                                          [[1, nn]])
                            nc.tensor.matmul(
                                _ap(ps, j * 512, [[1, nn]]), wsl, rhs,
                                start=(tt == 0), stop=(tt == 8))
                    dwt = [pbr.tile([128, 384], BF, tag=f"dw{t}{j}",
                                    name=f"dw{t}{j}", bufs=2)
                           for j in range(len(grp))]
                    for j, (r0, nr) in enumerate(grp):
                        nc.scalar.activation(
                            dwt[j][:, 0:nr * 128],
                            _ap(ps, j * 512, [[PST, nr], [1, 128]]),
                            AF.Relu,
                            bias=bia_s[:, B_DW + t:B_DW + t + 1])
                    dwg.append(dwt)
                for t in range(DW_TENSOR_GROUPS, 4):
                    slc = []
                    for j, (r0, nr) in enumerate(grp):
                        b0, dwc = dve_dw[t][r0 // 12]
                        slc.append(dwc[:, (r0 - b0) * 128:
                                       (r0 - b0 + nr) * 128])
                    dwg.append(slc)
                # pointwise
                for mt in range(2):
                    ps = pbmm.tile([128, 1024], F32, tag="bmm", name="pwm")
                    for kt in range(4):
                        wsl = wpw_s[:, kt * 256 + mt * 128:
                                    kt * 256 + mt * 128 + 128]
                        for j, (r0, nr) in enumerate(grp):
                            rhs = (dwg[kt][j][:, 0:nr * 128]
                                   if kt < DW_TENSOR_GROUPS else dwg[kt][j])
                            nc.tensor.matmul(
                                ps[:, j * 512:j * 512 + nr * 128], wsl,
                                rhs, start=(kt == 0), stop=(kt == 3))
                    qo = pbr.tile([128, 768], BF, tag="qo", bufs=1)
                    if len(grp) == 2:
                        src = _ap(ps, 0, [[512, 2], [1, grp[0][1] * 128]])
                    else:
                        src = _ap(ps, 0, [[1, grp[0][1] * 128]])
                    nc.scalar.activation(
                        qo[:, 0:nrows * 128], src, AF.Identity,
                        bias=bia_s[:, B_PW + mt:B_PW + mt + 1])
                    nc.sync.dma_start(
                        qo_dram[mt, :, r0g * 128:(r0g + nrows) * 128],
                        qo[:, 0:nrows * 128])

                # interleave the final gating once its qo rows are written
                pi = g0 // 2
                for bg in range(8):
                    if (16 * bg + 15) // 6 == pi:
                        emit_c2a(bg)

            pe2_cm.__exit__(None, None, None)
            pbmm_cm.__exit__(None, None, None)

    nc.compile()
    return nc


def _interp_matrix():
    s, n = 16, 128
    src = np.clip((np.arange(n) + 0.5) * (s / n) - 0.5, 0.0, s - 1.0)
    i0 = np.floor(src).astype(np.int64)
    i1 = np.minimum(i0 + 1, s - 1)
    w = src - i0
    M = np.zeros((s, n), np.float64)
    np.add.at(M, (i0, np.arange(n)), 1.0 - w)
    np.add.at(M, (i1, np.arange(n)), w)
    return M


def _bf(x):
    return np.ascontiguousarray(np.asarray(x, np.float32).astype(
        ml_dtypes.bfloat16))


def prep_consts(inputs):
    """Host-side layout prep of all weight tensors (shared across cores)."""
    f = {k: np.asarray(v, np.float32) for k, v in inputs.items()}

    w3 = f["w_ccam_b"]                      # [256, 128, 3, 3]
    w3t = np.zeros((128, 9 * 256), np.float32)
    for ky in range(3):
        for kx in range(3):
            t9 = ky * 3 + kx
            w3t[:, t9 * 256:(t9 + 1) * 256] = w3[:, :, ky, kx].T
    wenc = np.zeros((128, 32), np.float32)  # w_enc [16, 256]
    for half in range(2):
        wenc[:, half * 16:(half + 1) * 16] = \
            f["w_enc"][:, half * 128:(half + 1) * 128].T

    def pack_lhsT(wm, nt):
        # wm [out, in]; returns [128, 2*nt*128]: [ci, (kh*nt+mt)*128+co]
        o, cin = wm.shape
        r = np.zeros((128, 2 * nt * 128), np.float32)
        for kh in range(2):
            for mt in range(nt):
                r[:, (kh * nt + mt) * 128:(kh * nt + mt) * 128 + 128] = \
                    wm[mt * 128:(mt + 1) * 128,
                       kh * 128:(kh + 1) * 128].T
        return r

    wq_p = pack_lhsT(f["w_q"], 1)
    wk_p = pack_lhsT(f["w_k"], 1)
    wv_p = pack_lhsT(f["w_v"], 2)
    wrow_p = pack_lhsT(f["w_row"], 2)
    wcol_p = pack_lhsT(f["w_col"], 2)
    wproj_p = pack_lhsT(f["w_proj"], 2)

    wpw_p = np.zeros((128, 4 * 256), np.float32)   # w_pw [256, 512]
    for kt in range(4):
        for mt in range(2):
            wpw_p[:, kt * 256 + mt * 128:kt * 256 + mt * 128 + 128] = \
                f["w_pw"][mt * 128:(mt + 1) * 128,
                          kt * 128:(kt + 1) * 128].T

    dwdg = np.zeros((128, 36 * 128), np.float32)   # w_dw [512,1,3,3]
    ii = np.arange(128)
    for t in range(4):
        for tap9 in range(9):
            ky, kx = divmod(tap9, 3)
            dwdg[ii, (t * 9 + tap9) * 128 + ii] = \
                f["w_dw"][t * 128 + ii, 0, ky, kx]

    post_p = np.zeros((16, 4 * 512), np.float32)
    for pidx, nm in enumerate(["pos_rowq", "pos_rowk", "pos_colq", "pos_colk"]):
        p = f[nm]                                   # [4, 128, 16]
        for i in range(4):
            post_p[:, (pidx * 4 + i) * 128:(pidx * 4 + i) * 128 + 128] = \
                p[i].T                              # [16, 128]

    biases = np.zeros((128, 20), np.float32)
    biases[:, B_CCAM + 0] = f["b_ccam_b"][:128]
    biases[:, B_CCAM + 1] = f["b_ccam_b"][128:]
    biases[:16, B_ENC] = f["b_enc"]
    biases[:, B_Q] = f["b_q"]
    biases[:, B_K] = f["b_k"]
    biases[:, B_V + 0] = f["b_v"][:128]
    biases[:, B_V + 1] = f["b_v"][128:]
    for t in range(4):
        biases[:, B_DW + t] = f["b_dw"][t * 128:(t + 1) * 128]
    biases[:, B_PW + 0] = f["b_pw"][:128]
    biases[:, B_PW + 1] = f["b_pw"][128:]
    biases[:, B_ROW + 0] = f["b_row"][:128]
    biases[:, B_ROW + 1] = f["b_row"][128:]
    biases[:, B_COL + 0] = f["b_col"][:128]
    biases[:, B_COL + 1] = f["b_col"][128:]
    biases[:, B_PROJ3 + 0] = f["b_proj"][:128] + 3.0
    biases[:, B_PROJ3 + 1] = f["b_proj"][128:] + 3.0

    dwsc_p = np.zeros((128, 36), np.float32)
    for t in range(4):
        for tap9 in range(9):
            ky, kx = divmod(tap9, 3)
            dwsc_p[:, t * 9 + tap9] = f["w_dw"][t * 128:(t + 1) * 128,
                                                0, ky, kx]
    return {
        "dwsc": np.ascontiguousarray(dwsc_p),
        "w3t": _bf(w3t), "wenc": _bf(wenc),
        "wq": _bf(wq_p), "wk": _bf(wk_p), "wv": _bf(wv_p),
        "wqs": _bf(wq_p / 32.0), "wks": _bf(wk_p / 32.0),
        "wvs": _bf(wv_p / 32.0),
        "dwd": _bf(dwdg), "wpw": _bf(wpw_p),
        "wrow": _bf(wrow_p), "wcol": _bf(wcol_p), "wproj": _bf(wproj_p),
        "post": _bf(post_p), "interpm": _bf(_interp_matrix()),
        "identb": _bf(np.eye(128)),
        "identf": np.eye(128, dtype=np.float32),
        "onesb": _bf(np.ones((128, 1))),
        "biases": np.ascontiguousarray(biases),
    }


def kernel(**inputs) -> np.ndarray:
    x = np.asarray(inputs["x"], np.float32)          # [8, 128, 128, 128]
    scale = float(np.asarray(inputs["scale_ccam"]).reshape(-1)[0])

    key = round(scale, 9)
    if key not in _CACHE:
        _CACHE[key] = build_graph(scale)
    nc = _CACHE[key]

    consts = prep_consts(inputs)
    in_maps = []
    for core in range(8):
        m = dict(consts)
        m["xb"] = np.ascontiguousarray(x[core].reshape(128, N))
        in_maps.append(m)

    res = run_bass_kernel_spmd(nc, in_maps, core_ids=list(range(8)))
    outs = [res.results[i]["out"].reshape(256, 128, 128) for i in range(8)]
    return np.stack(outs).astype(np.float32)


if __name__ == "__main__":
    rng = np.random.default_rng(0)
    demo = {"x": rng.standard_normal((8, 128, 128, 128), dtype=np.float32)}
    print("kernel module OK")


# revision 36
# speedup vs baseline: 1.1855x; 1.1087x over previous
"""Trainium2 Bass kernel for nn_Align_54279796687162 (sparse_attention).

Pure data parallel: one sample per NeuronCore (B=8 over 8 cores).
Per-core layout: activations channel-major [C(partitions), n = h*128 + w].
bf16 matmul inputs, f32 PSUM accumulation.

v2 structure:
 - Phase A: conv3x3 in 12-row slabs; cb^T / cf^T via DMA-transpose (xbar);
   energy accumulated from transposed tiles; shunts of cb/cf computed here
   (DVE idle during conv); softmax -> attn halves -> on-device rank-16
   correction matrices A = scale*(W @ attn) so xf is never materialized.
 - Region 2 (one scheduling scope): qkv from cb + A@cf corrections (q/k
   spilled to DRAM, reloaded as padded windows for the depthwise conv),
   depthwise 3x3 (groups split tensor/DVE), pointwise, axial attention,
   final gating - all interleaved by Tile.

Self-contained: hardcodes shapes, builds the Bass/Tile graph, shards inputs,
runs via run_bass_kernel_spmd on cores 0-7, gathers the full output.
"""

import numpy as np
import ml_dtypes

import concourse.bass as bass
import concourse.mybir as mybir
import concourse.tile as tile
from concourse import bacc
from concourse.bass_utils import run_bass_kernel_spmd

BF = mybir.dt.bfloat16
F32 = mybir.dt.float32
AF = mybir.ActivationFunctionType
ALU = mybir.AluOpType
AX = mybir.AxisListType

H = W = 128
N = H * W            # 16384
BL = 512             # block size (4 rows * 128)
CH = 4               # chunks
SCALE = 0.25         # KD ** -0.5
PST = 132            # padded row stride for q/k/v (DW conv layout)
PSZ = PST * 130      # padded tensor size per partition
SLAB = 12            # conv slab rows

# depthwise groups 0..DW_TENSOR_GROUPS-1 (of q,k,v0,v1) run as diag matmuls
# on the tensor engine; the rest run as DVE FMA chains.
DW_TENSOR_GROUPS = 3

# bias column map in the packed [128, 20] f32 bias tile
B_CCAM, B_ENC, B_Q, B_K, B_V, B_DW, B_PW, B_ROW, B_COL, B_PROJ3 = (
    0, 2, 3, 4, 5, 7, 11, 13, 15, 17)

_CACHE = {}


def _ap(base, extra_off, free_dims):
    """Build an AP from a tile's base AP with custom free dims."""
    b = base[:]
    return bass.AP(b.tensor, b.offset + extra_off, [list(b.ap[0])] + free_dims)


def build_graph(scale_ccam: float):
    nc = bacc.Bacc(None, target_bir_lowering=False)

    xb = nc.dram_tensor("xb", [128, N], F32, kind="ExternalInput")
    w3t = nc.dram_tensor("w3t", [128, 9 * 256], BF, kind="ExternalInput")
    wenc = nc.dram_tensor("wenc", [128, 32], BF, kind="ExternalInput")
    wq = nc.dram_tensor("wq", [128, 256], BF, kind="ExternalInput")
    wk = nc.dram_tensor("wk", [128, 256], BF, kind="ExternalInput")
    wv = nc.dram_tensor("wv", [128, 512], BF, kind="ExternalInput")
    wqs = nc.dram_tensor("wqs", [128, 256], BF, kind="ExternalInput")
    wks = nc.dram_tensor("wks", [128, 256], BF, kind="ExternalInput")
    wvs = nc.dram_tensor("wvs", [128, 512], BF, kind="ExternalInput")
    dwd = nc.dram_tensor("dwd", [128, 36 * 128], BF, kind="ExternalInput")
    wpw = nc.dram_tensor("wpw", [128, 4 * 256], BF, kind="ExternalInput")
    wrow = nc.dram_tensor("wrow", [128, 512], BF, kind="ExternalInput")
    wcol = nc.dram_tensor("wcol", [128, 512], BF, kind="ExternalInput")
    wproj = nc.dram_tensor("wproj", [128, 512], BF, kind="ExternalInput")
    post = nc.dram_tensor("post", [16, 4 * 512], BF, kind="ExternalInput")
    interpm = nc.dram_tensor("interpm", [16, 128], BF, kind="ExternalInput")
    identb = nc.dram_tensor("identb", [128, 128], BF, kind="ExternalInput")
    identf = nc.dram_tensor("identf", [128, 128], F32, kind="ExternalInput")
    onesb = nc.dram_tensor("onesb", [128, 1], BF, kind="ExternalInput")
    biases = nc.dram_tensor("biases", [128, 20], F32, kind="ExternalInput")
    dwsc = nc.dram_tensor("dwsc", [128, 36], F32, kind="ExternalInput")

    cb_dram = nc.dram_tensor("cb_dram", [2, 128, N], BF, kind="Internal")
    cf_dram = nc.dram_tensor("cf_dram", [16, N], BF, kind="Internal")
    qk_dram = nc.dram_tensor("qk_dram", [2, 128, N], BF, kind="Internal")
    qo_dram = nc.dram_tensor("qo_dram", [2, 128, N], BF, kind="Internal")
    out = nc.dram_tensor("out", [256, N], F32, kind="ExternalOutput")

    # conv slab row-starts: 10 slabs of 12 rows + 1 slab of 8
    slabs = [(s * SLAB, SLAB) for s in range(10)] + [(120, 8)]
    taps = [(1, 1), (0, 1), (2, 1), (1, 0), (1, 2),
            (0, 0), (0, 2), (2, 0), (2, 2)]

    with tile.TileContext(nc) as tc:
      with tc.tile_pool(name="cst", bufs=1) as cst:
        wenc_s = cst.tile([128, 32], BF)
        wq_s = cst.tile([128, 256], BF)
        wk_s = cst.tile([128, 256], BF)
        wv_s = cst.tile([128, 512], BF)
        wqs_s = cst.tile([128, 256], BF)
        wks_s = cst.tile([128, 256], BF)
        wvs_s = cst.tile([128, 512], BF)
        wpw_s = cst.tile([128, 4 * 256], BF)
        wrow_s = cst.tile([128, 512], BF)
        wcol_s = cst.tile([128, 512], BF)
        wproj_s = cst.tile([128, 512], BF)
        post_s = cst.tile([16, 4 * 512], BF)
        interp_s = cst.tile([16, 128], BF)
        idb_s = cst.tile([128, 128], BF)
        ones_s = cst.tile([128, 1], BF)
        bia_s = cst.tile([128, 20], F32)
        dwsc_s = cst.tile([128, 36], F32)
        for t, d in [(wenc_s, wenc), (wq_s, wq), (wk_s, wk),
                     (wv_s, wv), (wqs_s, wqs), (wks_s, wks), (wvs_s, wvs),
                     (wpw_s, wpw), (wrow_s, wrow),
                     (wcol_s, wcol), (wproj_s, wproj), (post_s, post),
                     (interp_s, interpm), (idb_s, identb),
                     (ones_s, onesb), (bia_s, biases), (dwsc_s, dwsc)]:
            nc.sync.dma_start(t[:], d[:])

        # persistent small tensors produced in phase A, consumed later
        xfs_row = [cst.tile([128, 512], BF, tag=f"xfsr{h}", name=f"xfsr{h}")
                   for h in range(2)]
        xfs_col = [cst.tile([128, 512], F32, tag=f"xfsc{h}", name=f"xfsc{h}")
                   for h in range(2)]
        cfs_row = cst.tile([16, 512], F32)
        cfs_col = cst.tile([16, 512], F32)
        cfs_row_b = cst.tile([16, 512], BF)
        cfs_col_b = cst.tile([16, 512], BF)
        at_h = [cst.tile([128, 16], BF, tag=f"at{h}", name=f"at{h}")
                for h in range(2)]
        # correction matrices A^T [16, 128]: q, k, v0, v1 (normal + shunt).
        # normal set packed at partition offsets 32*i for tile_position use.
        A4n = cst.tile([128, 128], BF)
        A_s = [cst.tile([16, 128], BF, tag=f"As{i}", name=f"As{i}")
               for i in range(4)]
        xproj = {(d_, t_): cst.tile([128, 512], BF, tag=f"xp{d_}{t_}",
                                    name=f"xp{d_}{t_}")
                 for d_ in range(2) for t_ in range(2)}

        # =========================================================
        # Phase A: conv3x3 slabs; cb^T/cf^T via DMA transpose;
        # energy; shunts; softmax; A matrices
        # =========================================================
        with (
            tc.tile_pool(name="pa", bufs=1) as pa,
            tc.tile_pool(name="pasl", bufs=3) as pasl,
            tc.tile_pool(name="par", bufs=3) as par,
            tc.tile_pool(name="pamm", bufs=2, space="PSUM") as pamm,
            tc.tile_pool(name="pacf", bufs=2, space="PSUM") as pacf,
            tc.tile_pool(name="pae", bufs=1, space="PSUM") as pae,
            tc.tile_pool(name="pasm", bufs=1, space="PSUM") as pasm,
        ):
            xpad = pa.tile([128, 130 * 130], BF)
            w3_s = pa.tile([128, 9 * 256], BF)
            idf_s = pa.tile([128, 128], F32)
            nc.sync.dma_start(w3_s[:], w3t[:])
            nc.sync.dma_start(idf_s[:], identf[:])

            # pad borders only; interior filled by strided cast-DMA
            nc.vector.memset(_ap(xpad, 0, [[1, 130]]), 0.0)
            nc.vector.memset(_ap(xpad, 129 * 130, [[1, 130]]), 0.0)
            nc.vector.memset(_ap(xpad, 129, [[130, 129], [1, 2]]), 0.0)
            for rc in range(4):
                nc.gpsimd.dma_start(
                    _ap(xpad, 131 + rc * 32 * 130, [[130, 32], [1, 128]]),
                    xb[:, rc * 4096:(rc + 1) * 4096])

            e_ps = pae.tile([16, 256], F32)

            first_mm = [True]
            pend = []   # (cbT, cfT, srows) pending energy MMs, 1-slab delay

            def emit_energy(last):
                cbT, cfT, srows = pend.pop(0)
                for j in range(srows):
                    for half in range(2):
                        nc.tensor.matmul(
                            e_ps[:, half * 128:(half + 1) * 128],
                            cfT[:, j, :], cbT[half][:, j, :],
                            start=first_mm[0],
                            stop=(last and j == srows - 1 and half == 1))
                        first_mm[0] = False

            for si, (s0, srows) in enumerate(slabs):
                # --- conv3x3 for this slab, both halves ---
                grp = [(r0, min(3, s0 + srows - r0))
                       for r0 in range(s0, s0 + srows, 3)]
                cbs = [pasl.tile([128, SLAB * 128], BF, tag=f"cbs{h}",
                                 name=f"cbs{h}") for h in range(2)]
                for half in range(2):
                    for g0 in range(0, len(grp), 2):
                        pg = grp[g0:g0 + 2]
                        ps = pamm.tile([128, 1024], F32, tag="amm")
                        for t9 in range(9):
                            ky, kx = divmod(t9, 3)
                            for j, (r0, nr) in enumerate(pg):
                                rhs = _ap(xpad, (r0 + ky) * 130 + kx,
                                          [[1, nr * 130 - 2]])
                                nc.tensor.matmul(
                                    _ap(ps, j * 512, [[1, nr * 130 - 2]]),
                                    w3_s[:, t9 * 256 + half * 128:
                                         t9 * 256 + half * 128 + 128],
                                    rhs, start=(t9 == 0), stop=(t9 == 8))
                        nrows = sum(nr for _, nr in pg)
                        if len(pg) == 2:
                            src = _ap(ps, 0, [[512, 2], [130, pg[0][1]],
                                              [1, 128]])
                        else:
                            src = _ap(ps, 0, [[130, pg[0][1]], [1, 128]])
                        lr = pg[0][0] - s0
                        nc.scalar.activation(
                            cbs[half][:, lr * 128:(lr + nrows) * 128],
                            src, AF.Relu,
                            bias=bia_s[:, B_CCAM + half:B_CCAM + half + 1])
                    nc.sync.dma_start(
                        cb_dram[half, :, s0 * 128:(s0 + srows) * 128],
                        cbs[half][:, 0:srows * 128])

                # --- cf for this slab: relu(wenc @ cb + b_enc) ---
                nbl = (srows * 128) // 512
                cfsl = pasl.tile([16, SLAB * 128], BF, tag="cfsl")
                for b in range(nbl):
                    ps = pacf.tile([16, 512], F32, tag="acf")
                    for half in range(2):
                        nc.tensor.matmul(
                            ps[:], wenc_s[:, half * 16:half * 16 + 16],
                            cbs[half][:, b * 512:(b + 1) * 512],
                            start=(half == 0), stop=(half == 1))
                    nc.scalar.activation(
                        cfsl[:, b * 512:(b + 1) * 512], ps[:], AF.Relu,
                        bias=bia_s[:16, B_ENC:B_ENC + 1])
                nc.sync.dma_start(
                    cf_dram[:, s0 * 128:(s0 + srows) * 128],
                    cfsl[:, 0:srows * 128])

                # --- transposes via DMA xbar ---
                cbT = [pasl.tile([128, SLAB, 128], BF, tag=f"cbT{h}",
                                 name=f"cbT{h}") for h in range(2)]
                cfT = pasl.tile([128, SLAB, 16], BF, tag="cfT")
                for half in range(2):
                    nc.sync.dma_start(cbT[half][:, 0:srows, :],
                                      cbs[half][:, 0:srows * 128],
                                      transpose=True)
                nc.sync.dma_start(cfT[:, 0:srows, :],
                                  cfsl[:, 0:srows * 128], transpose=True)

                # --- energy accumulation, pipelined one slab behind ---
                pend.append((cbT, cfT, srows))
                if si >= 1:
                    emit_energy(last=False)

                # --- shunts of cb (into xfs_*) and cf (into cfs_*) ---
                for b4 in range(srows // 4):
                    b = (s0 // 4) + b4
                    lr = b4 * 4
                    ci = b // 8
                    for half in range(2):
                        with nc.allow_low_precision(reason="bf16 shunt sums"):
                            src = _ap(cbs[half], lr * 128,
                                      [[1, 4], [128, 4], [4, 32]])
                            dst = _ap(xfs_row[half], 4 * b,
                                      [[128, 4], [1, 4]])
                            nc.vector.tensor_reduce(dst, src, axis=AX.X,
                                                    op=ALU.add)
                        part = par.tile([128, 128], F32, tag=f"cp{half}",
                                        name=f"cp{half}", bufs=2)
                        src = _ap(cbs[half], lr * 128, [[1, 128], [128, 4]])
                        nc.vector.tensor_reduce(part[:], src, axis=AX.X,
                                                op=ALU.add)
                        dstc = xfs_col[half][:, ci * 128:(ci + 1) * 128]
                        if b % 8 == 0:
                            nc.gpsimd.tensor_copy(dstc, part[:])
                        else:
                            nc.gpsimd.tensor_tensor(dstc, dstc, part[:],
                                                    ALU.add)
                    # cf shunts
                    src = _ap(cfsl, lr * 128, [[1, 4], [128, 4], [4, 32]])
                    dst = _ap(cfs_row, 4 * b, [[128, 4], [1, 4]])
                    nc.vector.tensor_reduce(dst, src, axis=AX.X, op=ALU.add)
                    partf = par.tile([16, 128], F32, tag="cpf", bufs=2)
                    src = _ap(cfsl, lr * 128, [[1, 128], [128, 4]])
                    nc.vector.tensor_reduce(partf[:], src, axis=AX.X,
                                            op=ALU.add)
                    dstc = cfs_col[:, ci * 128:(ci + 1) * 128]
                    if b % 8 == 0:
                        nc.gpsimd.tensor_copy(dstc, partf[:])
                    else:
                        nc.gpsimd.tensor_tensor(dstc, dstc, partf[:], ALU.add)

            emit_energy(last=True)
            nc.vector.tensor_copy(cfs_row_b[:], cfs_row[:])
            nc.vector.tensor_copy(cfs_col_b[:], cfs_col[:])

            # --- CCAM softmax: attn = softmax(-energy) over K=16 ---
            e_sb = pa.tile([16, 256], F32)
            nc.scalar.activation(e_sb[:], e_ps[:], AF.Copy)
            for half in range(2):
                tps = pasm.tile([128, 16], F32, tag="sm")
                nc.tensor.transpose(
                    tps[:], e_sb[:, half * 128:(half + 1) * 128],
                    idf_s[:16, :16])
                e_c = par.tile([128, 16], F32, tag="ec")
                nc.vector.tensor_copy(e_c[:], tps[:])
                mn = par.tile([128, 1], F32, tag="mn")
                nc.vector.tensor_reduce(mn[:], e_c[:], axis=AX.X, op=ALU.min)
                ex = par.tile([128, 16], F32, tag="ex")
                nc.scalar.activation(ex[:], e_c[:], AF.Exp,
                                     bias=mn[:], scale=-1.0)
                sm = par.tile([128, 1], F32, tag="smv")
                nc.vector.tensor_reduce(sm[:], ex[:], axis=AX.X, op=ALU.add)
                rc = par.tile([128, 1], F32, tag="rc")
                nc.vector.reciprocal(rc[:], sm[:])
                nc.vector.tensor_scalar(at_h[half][:], ex[:], rc[:],
                                        float(scale_ccam), ALU.mult, ALU.mult)

            # --- A^T matrices: A^T = at^T @ W^T (both halves accumulated)
            # normal set lands in A4n rows 32*i..32*i+15 (i: q,k,v0,v1)
            for (dsts, wt, nt) in [(("n", 0), wq_s, 1),
                                   (("n", 1), wk_s, 1),
                                   (("n", 2), wv_s, 2),
                                   ((A_s[0],), wqs_s, 1),
                                   ((A_s[1],), wks_s, 1),
                                   ((A_s[2], A_s[3]), wvs_s, 2)]:
                for mt in range(nt):
                    ps = pasm.tile([16, 128], F32, tag="sm")
                    for half in range(2):
                        nc.tensor.matmul(
                            ps[:], at_h[half][:],
                            wt[:, (half * nt + mt) * 128:
                               (half * nt + mt) * 128 + 128],
                            start=(half == 0), stop=(half == 1))
                    if dsts[0] == "n":
                        i4 = dsts[1] + mt
                        nc.scalar.activation(
                            A4n[32 * i4:32 * i4 + 16, :], ps[:], AF.Copy)
                    else:
                        nc.scalar.activation(dsts[mt][:], ps[:], AF.Copy)

        # =========================================================
        # Region 2: qkv, depthwise+pointwise, axial attn, final
        # =========================================================
        with (
            tc.tile_pool(name="pv", bufs=1) as pv,
            tc.tile_pool(name="pb", bufs=1) as pb,
            tc.tile_pool(name="pbr", bufs=3) as pbr,
        ):
            pqk_cm = tc.tile_pool(name="pqk", bufs=2, space="PSUM")
            pqk = pqk_cm.__enter__()
            v_sb = [pv.tile([128, PSZ], BF, tag=f"v{h}", name=f"v{h}")
                    for h in range(2)]
            for t_ in v_sb:
                # zero only the pad cells: rows 0/129, cols {0,1,130,131}
                nc.gpsimd.memset(_ap(t_, 0, [[129 * PST, 2], [1, PST]]), 0.0)
                nc.gpsimd.memset(
                    _ap(t_, PST, [[PST, 128], [130, 2], [1, 2]]), 0.0)

            # ---- qkv production: 8 slabs of 2048 cols ----
            # q/k go to DRAM (plain layout); v stays resident (padded).
            # each weight loaded once per 4 matmuls; the 4 rank-16
            # corrections run concurrently via tile_position row groups.
            for pr in range(8):
                cbi = [pbr.tile([128, 2048], BF, tag=f"cbi{h}",
                                name=f"cbi{h}", bufs=2) for h in range(2)]
                cfi = pbr.tile([128, 2048], BF, tag="cfi", bufs=2)
                sl = slice(pr * 2048, (pr + 1) * 2048)
                nc.sync.dma_start(cbi[0][:], cb_dram[0, :, sl])
                nc.sync.dma_start(cbi[1][:], cb_dram[1, :, sl])
                for i4 in range(4):
                    nc.sync.dma_start(cfi[32 * i4:32 * i4 + 16, :],
                                      cf_dram[:, sl])

                for (ti, wt, i4s, bc, nt) in [
                        (0, wq_s, (0,), B_Q, 1),
                        (1, wk_s, (1,), B_K, 1),
                        (2, wv_s, (2, 3), B_V, 2)]:
                    for mt in range(nt):
                        i4 = i4s[mt]
                        ps = pqk.tile([128, 2048], F32, tag="qmm")
                        for kh in range(2):
                            for j in range(4):
                                nc.tensor.matmul(
                                    ps[:, j * 512:(j + 1) * 512],
                                    wt[:, (kh * nt + mt) * 128:
                                       (kh * nt + mt) * 128 + 128],
                                    cbi[kh][:, j * 512:(j + 1) * 512],
                                    start=(kh == 0), stop=False)
                        for j in range(4):
                            nc.tensor.matmul(
                                ps[:, j * 512:(j + 1) * 512],
                                A4n[32 * i4:32 * i4 + 16, :],
                                cfi[32 * i4:32 * i4 + 16,
                                    j * 512:(j + 1) * 512],
                                start=False, stop=True,
                                tile_position=(32 * i4, 0))
                        if ti < 2:
                            qkst = pbr.tile([128, 2048], BF, tag="qkst",
                                            bufs=1)
                            nc.scalar.activation(
                                qkst[:], ps[:], AF.Identity,
                                bias=bia_s[:, bc + mt:bc + mt + 1])
                            nc.sync.dma_start(qk_dram[ti, :, sl], qkst[:])
                        else:
                            pdst = _ap(v_sb[mt], (16 * pr + 1) * PST + 2,
                                       [[PST, 16], [1, 128]])
                            nc.scalar.activation(
                                pdst, ps[:], AF.Identity,
                                bias=bia_s[:, bc + mt:bc + mt + 1])

            pqk_cm.__exit__(None, None, None)
            pbmm_cm = tc.tile_pool(name="pbmm", bufs=2, space="PSUM")
            pbmm = pbmm_cm.__enter__()
            pcm_cm = tc.tile_pool(name="pcm", bufs=1, space="PSUM")
            pcm = pcm_cm.__enter__()

            # ---- DVE depthwise FMA chains (fills V during qkv/C1) ----
            # per 24-row chunk: 9-tap STT chain into acc, then one
            # bias+relu extraction of the whole chunk (strips pads).
            dve_dw = {}  # t -> list of (c0, crows, chunk-output tile)

            def dve_chain(t, c0):
                vsrc = v_sb[t - 2]
                crows = min(24, 128 - c0)
                start = (c0 + 1) * PST + 2
                nn = crows * PST - 4
                acc = pbr.tile([128, 24 * PST], BF, tag=f"dacc{t}",
                               name=f"dacc{t}", bufs=1)
                acc_ap = _ap(acc, 0, [[1, nn]])
                nc.vector.tensor_scalar(
                    acc_ap, _ap(vsrc, start, [[1, nn]]),
                    dwsc_s[:, t * 9 + 4:t * 9 + 5], None, ALU.mult)
                for (ky, kx) in taps[1:]:
                    tap9 = ky * 3 + kx
                    delta = (ky - 1) * PST + (kx - 1)
                    src = _ap(vsrc, start + delta, [[1, nn]])
                    nc.vector.scalar_tensor_tensor(
                        acc_ap, src,
                        dwsc_s[:, t * 9 + tap9:t * 9 + tap9 + 1],
                        acc_ap, ALU.mult, ALU.add)
                return crows, acc

            def dve_extract(t, c0, crows, acc, outs):
                # extract in 12-row pieces (smaller resident footprint)
                for s12 in range(0, crows, 12):
                    rows = min(12, crows - s12)
                    dwc = pbr.tile([128, 12 * 128], BF, tag=f"dwc{t}",
                                   name=f"dwc{t}", bufs=2)
                    nc.vector.tensor_scalar(
                        dwc[:, 0:rows * 128],
                        _ap(acc, s12 * PST, [[PST, rows], [1, 128]]),
                        bia_s[:, B_DW + t:B_DW + t + 1], 0.0,
                        ALU.add, ALU.max)
                    outs.append((c0 + s12, dwc))

            for t in range(DW_TENSOR_GROUPS, 4):
                pend_dw = None
                outs = []
                for c0 in range(0, 128, 24):
                    crows, acc = dve_chain(t, c0)
                    if pend_dw is not None:
                        p0, pcr, pacc = pend_dw
                        dve_extract(t, p0, pcr, pacc, outs)
                    pend_dw = (c0, crows, acc)
                p0, pcr, pacc = pend_dw
                dve_extract(t, p0, pcr, pacc, outs)
                dve_dw[t] = outs

            # ---- C1 axial attention ----
            xfs_cb = [pb.tile([128, 512], BF, tag=f"xfcb{h}",
                              name=f"xfcb{h}") for h in range(2)]
            for hh in range(2):
                nc.gpsimd.tensor_copy(xfs_cb[hh][:], xfs_col[hh][:])
            for d_ in range(2):
                xfs = xfs_row if d_ == 0 else xfs_cb
                cfs_b = cfs_row_b if d_ == 0 else cfs_col_b
                qs_att = pb.tile([128, 512], BF, tag="qsa", bufs=2)
                ks_att = pb.tile([128, 512], BF, tag="ksa", bufs=2)
                vs_att = [pb.tile([128, 512], BF, tag=f"vsa{h}",
                                  name=f"vsa{h}", bufs=2) for h in range(2)]
                for (dst, wt, As_i, bc, nt, pidx) in [
                        ([qs_att], wqs_s, (0,), B_Q, 1, 2 * d_),
                        ([ks_att], wks_s, (1,), B_K, 1, 2 * d_ + 1),
                        (vs_att, wvs_s, (2, 3), B_V, 2, None)]:
                    for mt in range(nt):
                        ps = pcm.tile([128, 512], F32, tag="cmm")
                        for kh in range(2):
                            nc.tensor.matmul(
                                ps[:],
                                wt[:, (kh * nt + mt) * 128:
                                   (kh * nt + mt) * 128 + 128],
                                xfs[kh][:], start=(kh == 0), stop=False)
                        nc.tensor.matmul(ps[:], A_s[As_i[mt]][:], cfs_b[:],
                                         start=False, stop=(pidx is None))
                        if pidx is not None:
                            for i in range(CH):
                                nc.tensor.matmul(
                                    ps[:, i * 128:(i + 1) * 128],
                                    post_s[:, (pidx * 4 + i) * 128:
                                           (pidx * 4 + i) * 128 + 128],
                                    interp_s[:], start=False, stop=(i == 3))
                        nc.scalar.activation(
                            dst[mt][:], ps[:], AF.Identity,
                            bias=bia_s[:, bc + mt:bc + mt + 1])

                # repack q/k: 4 heads per 32-partition row group
                q_pack = pb.tile([128, 1024], BF, tag="qp", name="qp", bufs=2)
                k_pack = pb.tile([128, 1024], BF, tag="kp", name="kp", bufs=2)
                for g in range(8):
                    po, co = 32 * (g % 4), (g // 4) * 512
                    nc.sync.dma_start(
                        q_pack[po:po + 16, co:co + 512],
                        qs_att[g * 16:(g + 1) * 16, :])
                    nc.sync.dma_start(
                        k_pack[po:po + 16, co:co + 512],
                        ks_att[g * 16:(g + 1) * 16, :])

                # v^T per chunk: [128(pos), i, 256(ch2)]
                vt_s = pb.tile([128, 4, 256], BF, tag="vt", bufs=2)
                for i in range(CH):
                    for hh in range(2):
                        tp = pcm.tile([128, 128], BF, tag="lps")
                        nc.tensor.transpose(
                            tp[:], vs_att[hh][:, i * 128:(i + 1) * 128],
                            idb_s[:])
                        nc.scalar.activation(
                            vt_s[:, i, hh * 128:(hh + 1) * 128], tp[:],
                            AF.Copy)

                xpre = [pb.tile([128, 512], BF, tag=f"xpre{t}",
                                name=f"xpre{t}", bufs=2) for t in range(2)]
                for i in range(CH):
                    for th in range(2):
                        asm_ps = pcm.tile([128, 128], BF, tag="asm")
                        for gg in range(4):
                            g = th * 4 + gg
                            po = 32 * (g % 4)
                            co = (g // 4) * 512
                            sl_gi = slice(co + i * 128, co + i * 128 + 128)
                            l_ps = pcm.tile([128, 128], F32, tag="lps")
                            nc.tensor.matmul(l_ps[:],
                                             k_pack[po:po + 16, sl_gi],
                                             q_pack[po:po + 16, sl_gi],
                                             start=True, stop=True,
                                             tile_position=(po, 0))
                            e_t = pbr.tile([128, 128], BF, tag="et", bufs=2)
                            nc.scalar.activation(e_t[:], l_ps[:], AF.Exp,
                                                 scale=SCALE)
                            av_ps = pcm.tile([128, 33], F32, tag="av")
                            nc.tensor.matmul(
                                av_ps[:, 0:32], e_t[:],
                                vt_s[:, i, g * 32:(g + 1) * 32],
                                start=True, stop=False)
                            nc.tensor.matmul(av_ps[:, 32:33], e_t[:],
                                             ones_s[:], start=False,
                                             stop=True)
                            rcp = pbr.tile([128, 1], F32, tag="rcp")
                            nc.vector.reciprocal(rcp[:], av_ps[:, 32:33])
                            xrn = pbr.tile([128, 32], BF, tag="xrn")
                            nc.scalar.activation(xrn[:], av_ps[:, 0:32],
                                                 AF.Copy, scale=rcp[:])
                            nc.tensor.transpose(
                                asm_ps[gg * 32:(gg + 1) * 32, :], xrn[:],
                                idb_s[:], tile_position=(0, gg * 32))
                        nc.scalar.activation(
                            xpre[th][:, i * 128:(i + 1) * 128], asm_ps[:],
                            AF.Relu)

                wproj_d = wrow_s if d_ == 0 else wcol_s
                bcol = B_ROW if d_ == 0 else B_COL
                for mt in range(2):
                    ps = pcm.tile([128, 512], F32, tag="cmm")
                    for kh in range(2):
                        nc.tensor.matmul(
                            ps[:],
                            wproj_d[:, (kh * 2 + mt) * 128:
                                    (kh * 2 + mt) * 128 + 128],
                            xpre[kh][:], start=(kh == 0), stop=(kh == 1))
                    nc.scalar.activation(
                        xproj[(d_, mt)][:], ps[:], AF.Identity,
                        bias=bia_s[:, bcol + mt:bcol + mt + 1])

            pcm_cm.__exit__(None, None, None)
            pe2_cm = tc.tile_pool(name="pe2", bufs=2, space="PSUM")
            pe2 = pe2_cm.__enter__()

            # ---- depthwise 3x3 ----
            dwd_s = pb.tile([128, 36 * 128], BF)
            nc.sync.dma_start(dwd_s[:], dwd[:])
            dblk = [(r0, 3) for r0 in range(0, 126, 3)] + [(126, 2)]

            def dve_chunk_of(r0):
                return r0 // 24

            def emit_c2a(bg):
                # xx = relu(v + bcast(xrow) + bcast(xcol));
                # att = hsig(proj(xx) + b + 3); out = att * qkv2
                xxg = []
                for j in range(4):
                    b = bg * 4 + j
                    xxr = []
                    for half in range(2):
                        xx = pbr.tile([128, BL], BF, tag=f"xx{half}",
                                      name=f"xx{half}", bufs=3)
                        rap = _ap(xproj[(0, half)], b * 16,
                                  [[1, 16], [0, 32]])
                        cap = _ap(xproj[(1, half)], (b // 2) * 32,
                                  [[0, 4], [0, 4], [1, 32]])
                        nc.vector.tensor_tensor(xx[:], rap, cap, ALU.add)
                        vap = _ap(v_sb[half], (4 * b + 1) * PST + 2,
                                  [[PST, 4], [1, 128]])
                        nc.vector.tensor_tensor(xx[:], xx[:], vap, ALU.add)
                        nc.vector.tensor_scalar(xx[:], xx[:], 0.0, None,
                                                ALU.max)
                        xxr.append(xx)
                    xxg.append(xxr)
                for mt in range(2):
                    for jp in range(2):
                        ps = pe2.tile([128, 1024], F32, tag="jps",
                                      name="jps")
                        for kh in range(2):
                            wsl = wproj_s[:, (kh * 2 + mt) * 128:
                                          (kh * 2 + mt) * 128 + 128]
                            for jj in range(2):
                                j = jp * 2 + jj
                                nc.tensor.matmul(
                                    ps[:, jj * 512:(jj + 1) * 512],
                                    wsl, xxg[j][kh][:],
                                    start=(kh == 0), stop=(kh == 1))
                        for jj in range(2):
                            j = jp * 2 + jj
                            b = bg * 4 + j
                            sl = slice(b * BL, (b + 1) * BL)
                            psj = ps[:, jj * 512:(jj + 1) * 512]
                            hs = pbr.tile([128, BL], BF, tag="hs", bufs=2)
                            nc.scalar.activation(
                                hs[:], psj, AF.Relu,
                                bias=bia_s[:, B_PROJ3 + mt:B_PROJ3 + mt + 1])
                            att_t = pbr.tile([128, BL], BF, tag="att",
                                             bufs=2)
                            nc.vector.tensor_scalar(
                                att_t[:], hs[:], 6.0, 1.0 / 6.0,
                                ALU.min, ALU.mult)
                            qo_in = pbr.tile([128, BL], BF, tag="qoin",
                                             bufs=2)
                            nc.sync.dma_start(qo_in[:], qo_dram[mt, :, sl])
                            ob = pbr.tile([128, BL], BF, tag="ob", bufs=2)
                            nc.vector.tensor_tensor(ob[:], att_t[:],
                                                    qo_in[:], ALU.mult)
                            nc.gpsimd.dma_start(
                                out[mt * 128:(mt + 1) * 128, sl], ob[:])

            # tensor groups + pointwise per 2-block group
            for g0 in range(0, len(dblk), 2):
                grp = dblk[g0:g0 + 2]
                r0g = grp[0][0]
                nrows = sum(nr for _, nr in grp)
                dwg = []
                # q/k windows: padded rows r0g .. r0g+nrows+1
                wins = []
                for t in range(min(DW_TENSOR_GROUPS, 2)):
                    win = pbr.tile([128, 8 * PST], BF, tag=f"win{t}",
                                   name=f"win{t}", bufs=2)
                    wrows = nrows + 2
                    # zero pad columns (and edge pad rows)
                    nc.gpsimd.memset(
                        _ap(win, 0, [[PST, wrows], [130, 2], [1, 2]]), 0.0)
                    ia = max(r0g - 1, 0)
                    ib = min(r0g + nrows, 127)
                    if r0g == 0:
                        nc.gpsimd.memset(_ap(win, 2, [[1, 128]]), 0.0)
                    if r0g + nrows > 127:
                        nc.gpsimd.memset(
                            _ap(win, (128 - r0g + 1) * PST + 2,
                                [[1, 128]]), 0.0)
                    nc.sync.dma_start(
                        _ap(win, (ia - (r0g - 1)) * PST + 2,
                            [[PST, ib - ia + 1], [1, 128]]),
                        qk_dram[t, :, ia * 128:(ib + 1) * 128])
                    wins.append(win)

                for t in range(DW_TENSOR_GROUPS):
                    ps = pbmm.tile([128, 1024], F32, tag="bmm")
                    for tt, (ky, kx) in enumerate(taps):
                        tap9 = ky * 3 + kx
                        wsl = dwd_s[:, (t * 9 + tap9) * 128:
                                    (t * 9 + tap9) * 128 + 128]
                        for j, (r0, nr) in enumerate(grp):
                            nn = nr * PST - 4
                            if t < 2:
                                rhs = _ap(wins[t],
                                          (r0 - r0g + ky) * PST + kx + 1,
                                          [[1, nn]])
                            else:
                                rhs = _ap(v_sb[t - 2],
                                          (r0 + ky) * PST + kx + 1,
                                          [[1, nn]])
                            nc.tensor.matmul(
                                _ap(ps, j * 512, [[1, nn]]), wsl, rhs,
                                start=(tt == 0), stop=(tt == 8))
                    dwt = [pbr.tile([128, 384], BF, tag=f"dw{t}{j}",
                                    name=f"dw{t}{j}", bufs=2)
                           for j in range(len(grp))]
                    for j, (r0, nr) in enumerate(grp):
                        nc.scalar.activation(
                            dwt[j][:, 0:nr * 128],
                            _ap(ps, j * 512, [[PST, nr], [1, 128]]),
                            AF.Relu,
                            bias=bia_s[:, B_DW + t:B_DW + t + 1])
                    dwg.append(dwt)
                for t in range(DW_TENSOR_GROUPS, 4):
                    slc = []
                    for j, (r0, nr) in enumerate(grp):
                        b0, dwc = dve_dw[t][r0 // 12]
                        slc.append(dwc[:, (r0 - b0) * 128:
                                       (r0 - b0 + nr) * 128])
                    dwg.append(slc)
                # pointwise
                for mt in range(2):
                    ps = pbmm.tile([128, 1024], F32, tag="bmm", name="pwm")
                    for kt in range(4):
                        wsl = wpw_s[:, kt * 256 + mt * 128:
                                    kt * 256 + mt * 128 + 128]
                        for j, (r0, nr) in enumerate(grp):
                            rhs = (dwg[kt][j][:, 0:nr * 128]
                                   if kt < DW_TENSOR_GROUPS else dwg[kt][j])
                            nc.tensor.matmul(
                                ps[:, j * 512:j * 512 + nr * 128], wsl,
                                rhs, start=(kt == 0), stop=(kt == 3))
                    qo = pbr.tile([128, 768], BF, tag="qo", bufs=1)
                    if len(grp) == 2:
                        src = _ap(ps, 0, [[512, 2], [1, grp[0][1] * 128]])
                    else:
                        src = _ap(ps, 0, [[1, grp[0][1] * 128]])
                    nc.scalar.activation(
                        qo[:, 0:nrows * 128], src, AF.Identity,
                        bias=bia_s[:, B_PW + mt:B_PW + mt + 1])
                    nc.sync.dma_start(
                        qo_dram[mt, :, r0g * 128:(r0g + nrows) * 128],
                        qo[:, 0:nrows * 128])

                # interleave the final gating once its qo rows are written
                pi = g0 // 2
                for bg in range(8):
                    if (16 * bg + 15) // 6 == pi:
                        emit_c2a(bg)

            pe2_cm.__exit__(None, None, None)
            pbmm_cm.__exit__(None, None, None)

    nc.compile()
    return nc


def _interp_matrix():
    s, n = 16, 128
    src = np.clip((np.arange(n) + 0.5) * (s / n) - 0.5, 0.0, s - 1.0)
    i0 = np.floor(src).astype(np.int64)
    i1 = np.minimum(i0 + 1, s - 1)
    w = src - i0
    M = np.zeros((s, n), np.float64)
    np.add.at(M, (i0, np.arange(n)), 1.0 - w)
    np.add.at(M, (i1, np.arange(n)), w)
    return M


def _bf(x):
    return np.ascontiguousarray(np.asarray(x, np.float32).astype(
        ml_dtypes.bfloat16))


def prep_consts(inputs):
    """Host-side layout prep of all weight tensors (shared across cores)."""
    f = {k: np.asarray(v, np.float32) for k, v in inputs.items()}

    w3 = f["w_ccam_b"]                      # [256, 128, 3, 3]
    w3t = np.zeros((128, 9 * 256), np.float32)
    for ky in range(3):
        for kx in range(3):
            t9 = ky * 3 + kx
            w3t[:, t9 * 256:(t9 + 1) * 256] = w3[:, :, ky, kx].T
    wenc = np.zeros((128, 32), np.float32)  # w_enc [16, 256]
    for half in range(2):
        wenc[:, half * 16:(half + 1) * 16] = \
            f["w_enc"][:, half * 128:(half + 1) * 128].T

    def pack_lhsT(wm, nt):
        # wm [out, in]; returns [128, 2*nt*128]: [ci, (kh*nt+mt)*128+co]
        o, cin = wm.shape
        r = np.zeros((128, 2 * nt * 128), np.float32)
        for kh in range(2):
            for mt in range(nt):
                r[:, (kh * nt + mt) * 128:(kh * nt + mt) * 128 + 128] = \
                    wm[mt * 128:(mt + 1) * 128,
                       kh * 128:(kh + 1) * 128].T
        return r

    wq_p = pack_lhsT(f["w_q"], 1)
    wk_p = pack_lhsT(f["w_k"], 1)
    wv_p = pack_lhsT(f["w_v"], 2)
    wrow_p = pack_lhsT(f["w_row"], 2)
    wcol_p = pack_lhsT(f["w_col"], 2)
    wproj_p = pack_lhsT(f["w_proj"], 2)

    wpw_p = np.zeros((128, 4 * 256), np.float32)   # w_pw [256, 512]
    for kt in range(4):
        for mt in range(2):
            wpw_p[:, kt * 256 + mt * 128:kt * 256 + mt * 128 + 128] = \
                f["w_pw"][mt * 128:(mt + 1) * 128,
                          kt * 128:(kt + 1) * 128].T

    dwdg = np.zeros((128, 36 * 128), np.float32)   # w_dw [512,1,3,3]
    ii = np.arange(128)
    for t in range(4):
        for tap9 in range(9):
            ky, kx = divmod(tap9, 3)
            dwdg[ii, (t * 9 + tap9) * 128 + ii] = \
                f["w_dw"][t * 128 + ii, 0, ky, kx]

    post_p = np.zeros((16, 4 * 512), np.float32)
    for pidx, nm in enumerate(["pos_rowq", "pos_rowk", "pos_colq", "pos_colk"]):
        p = f[nm]                                   # [4, 128, 16]
        for i in range(4):
            post_p[:, (pidx * 4 + i) * 128:(pidx * 4 + i) * 128 + 128] = \
                p[i].T                              # [16, 128]

    biases = np.zeros((128, 20), np.float32)
    biases[:, B_CCAM + 0] = f["b_ccam_b"][:128]
    biases[:, B_CCAM + 1] = f["b_ccam_b"][128:]
    biases[:16, B_ENC] = f["b_enc"]
    biases[:, B_Q] = f["b_q"]
    biases[:, B_K] = f["b_k"]
    biases[:, B_V + 0] = f["b_v"][:128]
    biases[:, B_V + 1] = f["b_v"][128:]
    for t in range(4):
        biases[:, B_DW + t] = f["b_dw"][t * 128:(t + 1) * 128]
    biases[:, B_PW + 0] = f["b_pw"][:128]
    biases[:, B_PW + 1] = f["b_pw"][128:]
    biases[:, B_ROW + 0] = f["b_row"][:128]
    biases[:, B_ROW + 1] = f["b_row"][128:]
    biases[:, B_COL + 0] = f["b_col"][:128]
    biases[:, B_COL + 1] = f["b_col"][128:]
    biases[:, B_PROJ3 + 0] = f["b_proj"][:128] + 3.0
    biases[:, B_PROJ3 + 1] = f["b_proj"][128:] + 3.0

    dwsc_p = np.zeros((128, 36), np.float32)
    for t in range(4):
        for tap9 in range(9):
            ky, kx = divmod(tap9, 3)
            dwsc_p[:, t * 9 + tap9] = f["w_dw"][t * 128:(t + 1) * 128,
                                                0, ky, kx]
    return {
        "dwsc": np.ascontiguousarray(dwsc_p),
        "w3t": _bf(w3t), "wenc": _bf(wenc),
        "wq": _bf(wq_p), "wk": _bf(wk_p), "wv": _bf(wv_p),
        "wqs": _bf(wq_p / 32.0), "wks": _bf(wk_p / 32.0),
        "wvs": _bf(wv_p / 32.0),
        "dwd": _bf(dwdg), "wpw": _bf(wpw_p),
        "wrow": _bf(wrow_p), "wcol": _bf(wcol_p), "wproj": _bf(wproj_p),
        "post": _bf(post_p), "interpm": _bf(_interp_matrix()),
        "identb": _bf(np.eye(128)),
        "identf": np.eye(128, dtype=np.float32),
        "onesb": _bf(np.ones((128, 1))),
        "biases": np.ascontiguousarray(biases),
    }


def kernel(**inputs) -> np.ndarray:
    x = np.asarray(inputs["x"], np.float32)          # [8, 128, 128, 128]
    scale = float(np.asarray(inputs["scale_ccam"]).reshape(-1)[0])

    key = round(scale, 9)
    if key not in _CACHE:
        _CACHE[key] = build_graph(scale)
    nc = _CACHE[key]

    consts = prep_consts(inputs)
    in_maps = []
    for core in range(8):
        m = dict(consts)
        m["xb"] = np.ascontiguousarray(x[core].reshape(128, N))
        in_maps.append(m)

    res = run_bass_kernel_spmd(nc, in_maps, core_ids=list(range(8)))
    outs = [res.results[i]["out"].reshape(256, 128, 128) for i in range(8)]
    return np.stack(outs).astype(np.float32)


if __name__ == "__main__":
    rng = np.random.default_rng(0)
    demo = {"x": rng.standard_normal((8, 128, 128, 128), dtype=np.float32)}
    print("kernel module OK")
